# revision 1
# baseline (speedup 1.0000x reference)
"""GatedSlotAttention2 Trainium2 Bass kernel.

Sharding: 2 heads per core x 8 cores (H=16). Each core computes its two
heads' full pipeline (projections -> short conv -> chunked gated-slot scan
-> RMSNorm-gate -> partial Wo matmul); host sums the 8 partial outputs.

Scan algorithm: chunk-parallel reformulation of the per-step recurrence
with chunk size C=64 (validated vs the sequential reference to ~6e-7 in
f32; bf16 projections give ~4e-3).
"""
import numpy as np
import ml_dtypes

import concourse.bass as bass
import concourse.bacc as bacc_mod
import concourse.mybir as mybir
import concourse.tile as tile
from concourse.bass_utils import run_bass_kernel_spmd

F32 = mybir.dt.float32
BF16 = mybir.dt.bfloat16
AF = mybir.ActivationFunctionType
ALU = mybir.AluOpType
MS = bass.MemorySpace

B, T, HID = 1, 1024, 2048
H, DK, DV, M, KW = 16, 128, 128, 128, 4
SCALE = DK ** -0.5
EPS = 1e-5
C = 64            # chunk length
NCH = T // C      # 16 chunks
NKT = HID // 128  # 16 contraction tiles
HL = 2            # heads per core

_CACHE = {}


def _build_nc():
    nc = bacc_mod.Bacc("TRN2")

    # ---------------- DRAM I/O ----------------
    d_xt = nc.dram_tensor("xt", [HID, T], BF16, kind="ExternalInput")        # X^T
    d_wq = nc.dram_tensor("wq", [HID, HL * DK], BF16, kind="ExternalInput")
    d_wk = nc.dram_tensor("wk", [HID, HL * DK], BF16, kind="ExternalInput")
    d_wv = nc.dram_tensor("wv", [HID, HL * DV], BF16, kind="ExternalInput")
    d_ww = nc.dram_tensor("ww", [HID, HL * M], BF16, kind="ExternalInput")
    d_wf1 = nc.dram_tensor("wf1", [HID, DV], BF16, kind="ExternalInput")
    d_wg1 = nc.dram_tensor("wg1", [HID, DV], BF16, kind="ExternalInput")
    d_wb = nc.dram_tensor("wb", [HID, HL], BF16, kind="ExternalInput")
    d_wf2 = nc.dram_tensor("wf2", [DV, HL * M], F32, kind="ExternalInput")
    d_wg2 = nc.dram_tensor("wg2", [DV, HL * DV], F32, kind="ExternalInput")
    d_bg2 = nc.dram_tensor("bg2", [1, HL * DV], F32, kind="ExternalInput")
    d_wo = nc.dram_tensor("wo", [HL * DV, HID], BF16, kind="ExternalInput")  # norm_w folded
    d_cq = nc.dram_tensor("cq", [128, HL, KW], F32, kind="ExternalInput")
    d_ck = nc.dram_tensor("ck", [128, HL, KW], F32, kind="ExternalInput")
    d_cv = nc.dram_tensor("cv", [128, HL, KW], F32, kind="ExternalInput")
    # constants
    d_trineg = nc.dram_tensor("trineg", [C, C], F32, kind="ExternalInput")       # -1 if j<=i
    d_trirev = nc.dram_tensor("trirev", [C, C], F32, kind="ExternalInput")       # -1 if j>i
    d_negc31 = nc.dram_tensor("negc31", [C, C], F32, kind="ExternalInput")       # -1 if j<=31
    d_maskS = nc.dram_tensor("masks", [C, C], F32, kind="ExternalInput")         # SCALE if j<=i
    d_maskJ = nc.dram_tensor("maskj", [C, C], mybir.dt.uint8, kind="ExternalInput")         # 1 if j<=i
    d_negones = nc.dram_tensor("negones", [C, 128], F32, kind="ExternalInput")   # all -1
    d_ident = nc.dram_tensor("ident", [128, 128], F32, kind="ExternalInput")
    d_ones1 = nc.dram_tensor("ones1", [1, C], F32, kind="ExternalInput")         # ones row

    d_out = nc.dram_tensor("out", [T, HID], F32, kind="ExternalOutput")

    with tile.TileContext(nc) as tc:
        with (
            tc.tile_pool(name="persist", bufs=1) as pp,
            tc.tile_pool(name="wpool", bufs=2) as wp,
            tc.tile_pool(name="convT", bufs=2) as cvp,
            tc.tile_pool(name="xpad", bufs=2) as xpp,
            tc.tile_pool(name="scr", bufs=2) as scr,
            tc.tile_pool(name="ps_proj", bufs=2, space=MS.PSUM) as ps_proj,
            tc.tile_pool(name="ps_scan", bufs=4, space=MS.PSUM) as ps_scan,
            tc.tile_pool(name="ps_out", bufs=2, space=MS.PSUM) as ps_out,
        ):
            # ---------- constants to SBUF ----------
            def load_const(dram, shape, dtype=F32):
                t = pp.tile(shape, dtype, tag=dram.name + "_sb")
                nc.sync.dma_start(t[:], dram[:])
                return t

            c_trineg = load_const(d_trineg, [C, C])
            c_trirev = load_const(d_trirev, [C, C])
            c_negc31 = load_const(d_negc31, [C, C])
            c_maskS = load_const(d_maskS, [C, C])
            c_maskJ = load_const(d_maskJ, [C, C], mybir.dt.uint8)
            c_negones = load_const(d_negones, [C, 128])
            c_ident = load_const(d_ident, [128, 128])
            c_ones1 = load_const(d_ones1, [1, C])
            c_wf2 = load_const(d_wf2, [DV, HL * M])
            c_wg2 = load_const(d_wg2, [DV, HL * DV])
            c_bg2 = load_const(d_bg2, [1, HL * DV])
            c_cq = load_const(d_cq, [128, HL, KW])
            c_ck = load_const(d_ck, [128, HL, KW])
            c_cv = load_const(d_cv, [128, HL, KW])
            c_eps6 = pp.tile([C, 1], F32, tag="c_eps6")
            nc.vector.memset(c_eps6[:], 1e-6)
            c_eps5 = pp.tile([C, 1], F32, tag="c_eps5")
            nc.vector.memset(c_eps5[:], EPS)

            # ---------- X^T stream tiles + big weights ----------
            xt_sb = pp.tile([128, NKT, T], BF16, tag="xt_sb")
            xtr = d_xt.rearrange("(k p) t -> k p t", p=128)
            for kt in range(NKT):
                nc.sync.dma_start(xt_sb[:, kt, :], xtr[kt])

            wo_sb = pp.tile([128, HL, HID], BF16, tag="wo_sb")
            wor = d_wo.rearrange("(h p) o -> h p o", p=128)
            for h in range(HL):
                nc.sync.dma_start(wo_sb[:, h, :], wor[h])

            # ---------- projections + conv + silu ----------
            # conv outputs, [channel, t] layout; q/k persist, v/w rotate
            qT = pp.tile([128, HL, T], F32, tag="qT")
            kT = pp.tile([128, HL, T], F32, tag="kT")

            def project_convT(d_w, c_cw, out_tile, name):
                """out[ct][c,t] = silu(conv1d(W[:,c].T @ X^T, cw)) per c-tile."""
                w_sb = wp.tile([128, NKT, HL * 128], BF16, tag="w_load")
                wr = d_w.rearrange("(k p) c -> k p c", p=128)
                for kt in range(NKT):
                    nc.sync.dma_start(w_sb[:, kt, :], wr[kt])
                for ct in range(HL):
                    acc = [None, None]
                    for tt in range(2):
                        ps = ps_proj.tile([128, 512], F32, tag="pp")
                        for kt in range(NKT):
                            nc.tensor.matmul(
                                ps[:],
                                w_sb[:, kt, ct * 128:(ct + 1) * 128],
                                xt_sb[:, kt, tt * 512:(tt + 1) * 512],
                                start=(kt == 0), stop=(kt == NKT - 1),
                            )
                        acc[tt] = ps
                    xpad = xpp.tile([128, T + KW - 1], F32, tag="xpad")
                    nc.vector.memset(xpad[:, 0:KW - 1], 0.0)
                    for tt in range(2):
                        nc.vector.tensor_copy(
                            xpad[:, KW - 1 + tt * 512: KW - 1 + (tt + 1) * 512],
                            acc[tt][:])
                    cacc = xpp.tile([128, T], F32, tag="convacc")
                    nc.vector.tensor_scalar_mul(
                        cacc[:], xpad[:, 0:T], c_cw[:, ct, 0:1])
                    for i in range(1, KW):
                        nc.vector.scalar_tensor_tensor(
                            cacc[:], xpad[:, i:i + T], c_cw[:, ct, i:i + 1],
                            cacc[:], op0=ALU.mult, op1=ALU.add)
                    se = xpp.tile([128, T], F32, tag="se")
                    nc.scalar.activation(se[:], cacc[:], AF.Exp, scale=-1.0)
                    nc.vector.tensor_scalar_add(se[:], se[:], 1.0)
                    nc.vector.reciprocal(se[:], se[:])
                    nc.vector.tensor_mul(out_tile[:, ct, :], cacc[:], se[:])

            project_convT(d_wq, c_cq, qT, "q")
            project_convT(d_wk, c_ck, kT, "k")
            vT = cvp.tile([128, HL, T], F32, tag="convT")
            project_convT(d_wv, c_cv, vT, "v")
            wT = cvp.tile([128, HL, T], F32, tag="convT")
            project_convT(d_ww, c_cv, wT, "w")

            # ---------- gate-path projections: F1T, G1T, betaT ----------
            def proj128T(d_w, tag):
                out = pp.tile([128, T], F32, tag=tag)
                w_sb = wp.tile([128, NKT, 128], BF16, tag="w_load")
                wr = d_w.rearrange("(k p) c -> k p c", p=128)
                for kt in range(NKT):
                    nc.sync.dma_start(w_sb[:, kt, :], wr[kt])
                for tt in range(2):
                    ps = ps_proj.tile([128, 512], F32, tag="pp")
                    for kt in range(NKT):
                        nc.tensor.matmul(
                            ps[:], w_sb[:, kt, :],
                            xt_sb[:, kt, tt * 512:(tt + 1) * 512],
                            start=(kt == 0), stop=(kt == NKT - 1))
                    nc.scalar.copy(out[:, tt * 512:(tt + 1) * 512], ps[:])
                return out

            f1T = proj128T(d_wf1, "f1T")
            g1T = proj128T(d_wg1, "g1T")

            betaT = pp.tile([HL, T], F32, tag="betaT")
            wb_sb = wp.tile([128, NKT, HL], BF16, tag="wb_load")
            wbr = d_wb.rearrange("(k p) c -> k p c", p=128)
            for kt in range(NKT):
                nc.sync.dma_start(wb_sb[:, kt, :], wbr[kt])
            for tt in range(2):
                ps = ps_proj.tile([HL, 512], F32, tag="pp")
                for kt in range(NKT):
                    nc.tensor.matmul(
                        ps[:], wb_sb[:, kt, :],
                        xt_sb[:, kt, tt * 512:(tt + 1) * 512],
                        start=(kt == 0), stop=(kt == NKT - 1))
                bsl = betaT[:, tt * 512:(tt + 1) * 512]
                nc.scalar.activation(bsl, ps[:], AF.Exp, scale=-1.0)
                nc.vector.tensor_scalar_add(bsl, bsl, 1.0)
                nc.vector.reciprocal(bsl, bsl)

            # ---------- states ----------
            Sk = [pp.tile([DK, M], F32, name=f"Sk{h}", tag=f"Sk{h}") for h in range(HL)]
            Sv = [pp.tile([M, DV], F32, name=f"Sv{h}", tag=f"Sv{h}") for h in range(HL)]
            for h in range(HL):
                nc.vector.memset(Sk[h][:], 0.0)
                nc.vector.memset(Sv[h][:], 0.0)

            oT = [pp.tile([DV, NCH, C], BF16, name=f"oT{h}", tag=f"oT{h}") for h in range(HL)]

            # ---------- chunked scan ----------
            for n in range(NCH):
                t0 = n * C
                # shared across the two heads: gpos/gate/beta for this chunk
                gps = ps_scan.tile([C, HL * M], F32, tag="ps")
                nc.tensor.matmul(gps[:], f1T[:, t0:t0 + C], c_wf2[:],
                                 start=True, stop=True)
                gpos = scr.tile([C, HL * M], F32, tag="gpos")
                nc.scalar.activation(gpos[:], gps[:], AF.Exp, scale=-1.0)
                nc.scalar.activation(gpos[:], gpos[:], AF.Ln, bias=1.0)

                gt_ps = ps_scan.tile([C, HL * DV], F32, tag="ps")
                nc.tensor.matmul(gt_ps[:], g1T[:, t0:t0 + C], c_wg2[:],
                                 start=True, stop=False)
                nc.tensor.matmul(gt_ps[:], c_ones1[:], c_bg2[:],
                                 start=False, stop=True)
                gate = scr.tile([C, HL * DV], F32, tag="gate")
                nc.scalar.activation(gate[:], gt_ps[:], AF.Exp, scale=-1.0)
                nc.vector.tensor_scalar_add(gate[:], gate[:], 1.0)
                nc.vector.reciprocal(gate[:], gate[:])

                bt_ps = ps_scan.tile([C, HL], F32, tag="ps")
                nc.tensor.transpose(bt_ps[:], betaT[:, t0:t0 + C],
                                    c_ident[0:HL, 0:HL])
                beta = scr.tile([C, HL], F32, tag="beta")
                nc.scalar.copy(beta[:], bt_ps[:])

                for h in range(HL):
                    hs = slice(h * 128, (h + 1) * 128)
                    # --- per-chunk transposes: K, V, W ---
                    kps = ps_scan.tile([C, 128], F32, tag="ps")
                    nc.tensor.transpose(kps[:], kT[:, h, t0:t0 + C], c_ident[:])
                    Kc = scr.tile([C, 128], F32, tag="Kc")
                    nc.scalar.copy(Kc[:], kps[:])

                    vps = ps_scan.tile([C, 128], F32, tag="ps")
                    nc.tensor.transpose(vps[:], vT[:, h, t0:t0 + C], c_ident[:])
                    Vc = scr.tile([C, 128], F32, tag="Vc")
                    nc.scalar.copy(Vc[:], vps[:])

                    wps = ps_scan.tile([C, 128], F32, tag="ps")
                    nc.tensor.transpose(wps[:], wT[:, h, t0:t0 + C], c_ident[:])
                    # l2norm + beta scaling -> bw
                    w2 = scr.tile([C, 128], F32, tag="w2")
                    ss = scr.tile([C, 1], F32, tag="ss")
                    nc.scalar.activation(w2[:], wps[:], AF.Square, accum_out=ss[:])
                    sd = scr.tile([C, 1], F32, tag="sd")
                    nc.scalar.activation(sd[:], ss[:], AF.Ln, bias=c_eps6[:])
                    rs = scr.tile([C, 1], F32, tag="rs")
                    nc.scalar.activation(rs[:], sd[:], AF.Exp, scale=-0.5)
                    rsb = scr.tile([C, 1], F32, tag="rsb")
                    nc.vector.tensor_mul(rsb[:], rs[:], beta[:, h:h + 1])
                    bw = scr.tile([C, 128], F32, tag="bw")
                    nc.vector.tensor_scalar_mul(bw[:], wps[:], rsb[:])

                    # --- gate cumsums (via triangular matmuls) ---
                    gsl = gpos[:, hs]
                    gc_ps = ps_scan.tile([C, M], F32, tag="ps")
                    nc.tensor.matmul(gc_ps[:], c_trineg[:], gsl,
                                     start=True, stop=True)
                    Gc = scr.tile([C, M], F32, tag="Gc")
                    nc.scalar.copy(Gc[:], gc_ps[:])
                    grev_ps = ps_scan.tile([C, M], F32, tag="ps")
                    nc.tensor.matmul(grev_ps[:], c_trirev[:], gsl,
                                     start=True, stop=True)
                    b1_ps = ps_scan.tile([C, M], F32, tag="ps")
                    nc.tensor.matmul(b1_ps[:], c_negc31[:], gsl,
                                     start=True, stop=True)
                    Gcp = scr.tile([C, M], F32, tag="Gcp")
                    nc.vector.tensor_sub(Gcp[:], Gc[:], b1_ps[:])
                    Lam = scr.tile([C, M], F32, tag="Lam")
                    nc.scalar.activation(Lam[:], Gc[:], AF.Exp)
                    Epos = scr.tile([C, M], F32, tag="Epos")
                    nc.scalar.activation(Epos[:], Gcp[:], AF.Exp)
                    Enege = scr.tile([C, M], F32, tag="Enege")
                    nc.scalar.activation(Enege[:], Gcp[:], AF.Exp, scale=-1.0)
                    Eneg = scr.tile([C, M], F32, tag="Eneg")
                    nc.vector.tensor_mul(Eneg[:], Enege[:], bw[:])
                    Ereve = scr.tile([C, M], F32, tag="Ereve")
                    nc.scalar.activation(Ereve[:], grev_ps[:], AF.Exp)
                    Kdec = scr.tile([C, M], F32, tag="Kdec")
                    nc.vector.tensor_mul(Kdec[:], Ereve[:], bw[:])

                    # chunk-end decay broadcasts
                    lcb_ps = ps_scan.tile([128, M], F32, tag="ps")
                    nc.tensor.matmul(lcb_ps[:], c_negones[:], gsl,
                                     start=True, stop=True)
                    LamCb = scr.tile([128, M], F32, tag="LamCb")
                    nc.scalar.activation(LamCb[:], lcb_ps[:], AF.Exp)
                    lcc_ps = ps_scan.tile([M, 1], F32, tag="ps")
                    nc.tensor.matmul(lcc_ps[:], gsl, c_negones[:, 0:1],
                                     start=True, stop=True)
                    LamCc = scr.tile([M, 1], F32, tag="LamCc")
                    nc.scalar.activation(LamCc[:], lcc_ps[:], AF.Exp)

                    # --- pass A: scores + softmax ---
                    pt_ps = ps_scan.tile([C, C], F32, tag="ps")
                    nc.tensor.matmul(pt_ps[:], kT[:, h, t0:t0 + C],
                                     qT[:, h, t0:t0 + C], start=True, stop=True)
                    Ptm = scr.tile([C, C], F32, tag="Ptm")
                    nc.vector.tensor_mul(Ptm[:], pt_ps[:], c_maskS[:])
                    intra_ps = ps_scan.tile([C, M], F32, tag="ps")
                    nc.tensor.matmul(intra_ps[:], Ptm[:], Eneg[:],
                                     start=True, stop=True)
                    qs_ps = ps_scan.tile([C, M], F32, tag="ps")
                    nc.tensor.matmul(qs_ps[:], qT[:, h, t0:t0 + C], Sk[h][:],
                                     start=True, stop=True)
                    s1 = scr.tile([C, M], F32, tag="s1")
                    nc.vector.scalar_tensor_tensor(
                        s1[:], qs_ps[:], SCALE, Lam[:],
                        op0=ALU.mult, op1=ALU.mult)
                    s2 = scr.tile([C, M], F32, tag="s2")
                    nc.vector.tensor_mul(s2[:], intra_ps[:], Epos[:])
                    sS = scr.tile([C, M], F32, tag="sS")
                    nc.vector.tensor_add(sS[:], s1[:], s2[:])
                    mx = scr.tile([C, 1], F32, tag="mx")
                    nc.vector.tensor_reduce(mx[:], sS[:], mybir.AxisListType.X,
                                            ALU.max)
                    nmx = scr.tile([C, 1], F32, tag="nmx")
                    nc.vector.tensor_scalar_mul(nmx[:], mx[:], -1.0)
                    pexp = scr.tile([C, M], F32, tag="pexp")
                    den = scr.tile([C, 1], F32, tag="den")
                    nc.scalar.activation(pexp[:], sS[:], AF.Exp, bias=nmx[:],
                                         accum_out=den[:])
                    rec = scr.tile([C, 1], F32, tag="rec")
                    nc.vector.reciprocal(rec[:], den[:])
                    aL = scr.tile([C, M], F32, tag="aL")
                    nc.vector.scalar_tensor_tensor(
                        aL[:], pexp[:], rec[:], Lam[:],
                        op0=ALU.mult, op1=ALU.mult)
                    aE = scr.tile([C, M], F32, tag="aE")
                    nc.vector.scalar_tensor_tensor(
                        aE[:], pexp[:], rec[:], Epos[:],
                        op0=ALU.mult, op1=ALU.mult)

                    # --- pass B: output ---
                    alt_ps = ps_scan.tile([M, C], F32, tag="ps")
                    nc.tensor.transpose(alt_ps[:], aL[:], c_ident[0:C, 0:C])
                    aLT = scr.tile([M, C], F32, tag="aLT")
                    nc.scalar.copy(aLT[:], alt_ps[:])
                    aet_ps = ps_scan.tile([M, C], F32, tag="ps")
                    nc.tensor.transpose(aet_ps[:], aE[:], c_ident[0:C, 0:C])
                    aET = scr.tile([M, C], F32, tag="aET")
                    nc.scalar.copy(aET[:], aet_ps[:])
                    ent_ps = ps_scan.tile([M, C], F32, tag="ps")
                    nc.tensor.transpose(ent_ps[:], Eneg[:], c_ident[0:C, 0:C])
                    EnegT = scr.tile([M, C], F32, tag="EnegT")
                    nc.scalar.copy(EnegT[:], ent_ps[:])

                    rt_ps = ps_scan.tile([C, C], F32, tag="ps")
                    nc.tensor.matmul(rt_ps[:], EnegT[:], aET[:],
                                     start=True, stop=True)
                    Rmt = scr.tile([C, C], F32, tag="Rmt")
                    nc.vector.memset(Rmt[:], 0.0)
                    nc.vector.copy_predicated(Rmt[:], c_maskJ[:], rt_ps[:])

                    o_ps = ps_scan.tile([C, DV], F32, tag="ps")
                    nc.tensor.matmul(o_ps[:], aLT[:], Sv[h][:],
                                     start=True, stop=False)
                    nc.tensor.matmul(o_ps[:], Rmt[:], Vc[:],
                                     start=False, stop=True)

                    # --- state updates ---
                    skk_ps = ps_scan.tile([DK, M], F32, tag="ps")
                    nc.tensor.matmul(skk_ps[:], Kc[:], Kdec[:],
                                     start=True, stop=True)
                    skt = scr.tile([DK, M], F32, tag="skt")
                    nc.vector.tensor_mul(skt[:], Sk[h][:], LamCb[:])
                    nc.vector.tensor_add(Sk[h][:], skt[:], skk_ps[:])
                    svk_ps = ps_scan.tile([M, DV], F32, tag="ps")
                    nc.tensor.matmul(svk_ps[:], Kdec[:], Vc[:],
                                     start=True, stop=True)
                    svt = scr.tile([M, DV], F32, tag="svt")
                    nc.vector.tensor_scalar_mul(svt[:], Sv[h][:], LamCc[:])
                    nc.vector.tensor_add(Sv[h][:], svt[:], svk_ps[:])

                    # --- epilogue: RMSNorm * sigmoid(gate), transpose ---
                    o2 = scr.tile([C, DV], F32, tag="o2")
                    oss = scr.tile([C, 1], F32, tag="oss")
                    nc.scalar.activation(o2[:], o_ps[:], AF.Square,
                                         accum_out=oss[:])
                    orm = scr.tile([C, 1], F32, tag="orm")
                    nc.scalar.activation(orm[:], oss[:], AF.Ln,
                                         scale=1.0 / DV, bias=c_eps5[:])
                    orr = scr.tile([C, 1], F32, tag="orr")
                    nc.scalar.activation(orr[:], orm[:], AF.Exp, scale=-0.5)
                    o1 = scr.tile([C, DV], F32, tag="o1")
                    nc.vector.tensor_mul(o1[:], o_ps[:], gate[:, hs])
                    of = scr.tile([C, DV], F32, tag="of")
                    nc.vector.tensor_scalar_mul(of[:], o1[:], orr[:])
                    ot_ps = ps_scan.tile([DV, C], F32, tag="ps")
                    nc.tensor.transpose(ot_ps[:], of[:], c_ident[0:C, 0:C])
                    nc.scalar.copy(oT[h][:, n, :], ot_ps[:])

            # ---------- output projection (partial; host sums cores) ----------
            for tt in range(8):
                for cl in range(4):
                    ps = ps_out.tile([128, 512], F32, tag="po")
                    for h in range(HL):
                        nc.tensor.matmul(
                            ps[:],
                            oT[h][:, 2 * tt:2 * tt + 2, :],
                            wo_sb[:, h, cl * 512:(cl + 1) * 512],
                            start=(h == 0), stop=(h == HL - 1))
                    osb = scr.tile([128, 512], F32, tag="outsb", bufs=3)
                    nc.scalar.copy(osb[:], ps[:])
                    nc.sync.dma_start(
                        d_out[tt * 128:(tt + 1) * 128, cl * 512:(cl + 1) * 512],
                        osb[:])
    nc.compile()
    return nc


def _host_inputs(inputs):
    """Build the 8 per-core input maps from the full-problem inputs."""
    f32 = np.float32
    bf16 = ml_dtypes.bfloat16
    X = np.ascontiguousarray(np.asarray(inputs["hidden_states"], f32)[0])  # [T, HID]
    XT = np.ascontiguousarray(X.T).astype(bf16)

    tri_neg = np.triu(np.full((C, C), -1.0, f32))          # [j,i] -1 if j<=i
    tri_rev = np.tril(np.full((C, C), -1.0, f32), -1)      # -1 if j>i
    negc31 = np.zeros((C, C), f32); negc31[:32, :] = -1.0  # -1 if j<=31
    maskS = np.triu(np.full((C, C), SCALE, f32))
    maskJ = np.triu(np.ones((C, C), f32))
    negones = np.full((C, 128), -1.0, f32)
    ident = np.eye(128, dtype=f32)
    ones1 = np.ones((1, C), f32)

    Wo_full = np.asarray(inputs["Wo"], f32) * np.tile(
        np.asarray(inputs["norm_w"], f32), H)[:, None]

    in_maps = []
    for c in range(8):
        hsl = slice(c * HL * 128, (c + 1) * HL * 128)
        bsl = slice(c * HL, (c + 1) * HL)
        m = {
            "xt": XT,
            "wq": np.asarray(inputs["Wq"], f32)[:, hsl].astype(bf16),
            "wk": np.asarray(inputs["Wk"], f32)[:, hsl].astype(bf16),
            "wv": np.asarray(inputs["Wv"], f32)[:, hsl].astype(bf16),
            "ww": np.asarray(inputs["Ww"], f32)[:, hsl].astype(bf16),
            "wf1": np.asarray(inputs["Wf1"], f32).astype(bf16),
            "wg1": np.asarray(inputs["Wg1"], f32).astype(bf16),
            "wb": np.asarray(inputs["Wb"], f32)[:, bsl].astype(bf16),
            "wf2": np.ascontiguousarray(np.asarray(inputs["Wf2"], f32)[:, hsl]),
            "wg2": np.ascontiguousarray(np.asarray(inputs["Wg2"], f32)[:, hsl]),
            "bg2": np.ascontiguousarray(
                np.asarray(inputs["bg2"], f32)[None, hsl]),
            "wo": np.ascontiguousarray(Wo_full[hsl]).astype(bf16),
            "cq": np.ascontiguousarray(
                np.asarray(inputs["cq"], f32)[hsl].reshape(HL, 128, KW)
                .transpose(1, 0, 2)),
            "ck": np.ascontiguousarray(
                np.asarray(inputs["ck"], f32)[hsl].reshape(HL, 128, KW)
                .transpose(1, 0, 2)),
            "cv": np.ascontiguousarray(
                np.asarray(inputs["cv"], f32)[hsl].reshape(HL, 128, KW)
                .transpose(1, 0, 2)),
            "trineg": tri_neg, "trirev": tri_rev, "negc31": negc31,
            "masks": maskS, "maskj": maskJ.astype(np.uint8), "negones": negones,
            "ident": ident, "ones1": ones1,
        }
        in_maps.append(m)
    return in_maps


def kernel(_trace=False, **inputs):
    if "nc" not in _CACHE:
        _CACHE["nc"] = _build_nc()
    nc = _CACHE["nc"]
    in_maps = _host_inputs(inputs)
    res = run_bass_kernel_spmd(nc, in_maps, core_ids=list(range(8)),
                               trace=_trace)
    _CACHE["last_result"] = res
    out = np.zeros((T, HID), np.float32)
    for r in res.results:
        out += r["out"]
    return out.reshape(B, T, HID)



# revision 24
# speedup vs baseline: 1.3346x; 1.3346x over previous
"""GatedSlotAttention2 Trainium2 Bass kernel (optimized).

Sharding: 2 heads per core x 8 cores (H=16). Each core runs the full
pipeline for its two heads (projections -> causal conv+silu -> chunked
gated-slot scan -> RMSNorm-gate -> partial Wo matmul); the host sums the
8 partial outputs.

Key optimizations over the first working version:
- Chunk length C=128 (fills all SBUF partitions, halves instruction count).
- Scalar engine uses only {Silu, Tanh, Square, Ln, Exp, Copy}, emitted
  grouped by function family so the activation table is loaded ~5 times
  total instead of per-op (sigmoid/softplus are computed via tanh+ln).
- State-independent work (gates, cumsum decays, transposes, intra-chunk
  matmuls, state-update outer products) is hoisted out of the serial scan.
- bf16 matmul inputs everywhere validated safe (4x faster PE rows), f32
  kept for cumsums and state accumulation.
- PSUM->SBUF traffic spread across Pool/Vector engines; Scalar does only
  activations.
"""
import numpy as np
import ml_dtypes

import concourse.bass as bass
import concourse.bacc as bacc_mod
import concourse.mybir as mybir
import concourse.tile as tile
from concourse.bass_utils import run_bass_kernel_spmd

F32 = mybir.dt.float32
F32R = mybir.dt.float32r
BF16 = mybir.dt.bfloat16
U8 = mybir.dt.uint8
AF = mybir.ActivationFunctionType
ALU = mybir.AluOpType
MS = bass.MemorySpace
AX = mybir.AxisListType

B, T, HID = 1, 1024, 2048
H, DK, DV, M, KW = 16, 128, 128, 128, 4
SCALE = DK ** -0.5
EPS = 1e-5
C = 128           # chunk length
NCH = T // C      # 8 chunks
NKT = HID // 128  # 16 contraction tiles
HL = 2            # heads per core

_CACHE = {}


def _build_nc():
    nc = bacc_mod.Bacc("TRN2")

    # ---------------- DRAM I/O ----------------
    d_xt = nc.dram_tensor("xt", [HID, T], BF16, kind="ExternalInput")
    d_wq = nc.dram_tensor("wq", [HID, HL * DK], BF16, kind="ExternalInput")
    d_wk = nc.dram_tensor("wk", [HID, HL * DK], BF16, kind="ExternalInput")
    d_wv = nc.dram_tensor("wv", [HID, HL * DV], BF16, kind="ExternalInput")
    d_ww = nc.dram_tensor("ww", [HID, HL * M], BF16, kind="ExternalInput")
    d_wf1 = nc.dram_tensor("wf1", [HID, DV], BF16, kind="ExternalInput")
    d_wg1 = nc.dram_tensor("wg1", [HID, DV], BF16, kind="ExternalInput")
    d_wb = nc.dram_tensor("wb", [HID, HL], BF16, kind="ExternalInput")
    d_wf2 = nc.dram_tensor("wf2", [DV, HL * M], F32, kind="ExternalInput")
    d_wg2 = nc.dram_tensor("wg2", [DV, HL * DV], F32, kind="ExternalInput")
    d_bg2 = nc.dram_tensor("bg2", [1, HL * DV], F32, kind="ExternalInput")
    d_wo = nc.dram_tensor("wo", [HL * DV, HID], BF16, kind="ExternalInput")
    d_cq = nc.dram_tensor("cq", [128, HL, KW], F32, kind="ExternalInput")
    d_ck = nc.dram_tensor("ck", [128, HL, KW], F32, kind="ExternalInput")
    d_cv = nc.dram_tensor("cv", [128, HL, KW], F32, kind="ExternalInput")
    # constants
    d_tripos = nc.dram_tensor("tripos", [C, C], F32, kind="ExternalInput")
    d_trimid = nc.dram_tensor("trimid", [C, C], F32, kind="ExternalInput")
    d_trirevs = nc.dram_tensor("trirevs", [C, C], F32, kind="ExternalInput")
    d_onescp = nc.dram_tensor("onescp", [C, 128], F32, kind="ExternalInput")
    d_onescol = nc.dram_tensor("onescol", [C, 1], F32, kind="ExternalInput")
    d_masks = nc.dram_tensor("masks", [C, C], F32, kind="ExternalInput")
    d_maskj = nc.dram_tensor("maskj", [C, C], U8, kind="ExternalInput")
    d_ident = nc.dram_tensor("ident", [128, 128], F32, kind="ExternalInput")
    d_identb = nc.dram_tensor("identb", [128, 128], BF16, kind="ExternalInput")
    d_ones1 = nc.dram_tensor("ones1", [1, C], F32, kind="ExternalInput")

    d_out = nc.dram_tensor("out", [T, HID], F32, kind="ExternalOutput")

    with tile.TileContext(nc) as tc:
        with (
            tc.tile_pool(name="persist", bufs=1) as pp,
            tc.tile_pool(name="scr", bufs=3) as scr,
            tc.tile_pool(name="scrB", bufs=3) as scrB,
            tc.tile_pool(name="psA", bufs=3, space=MS.PSUM) as psA,
            tc.tile_pool(name="psB", bufs=3, space=MS.PSUM) as psB,
            tc.tile_pool(name="psC", bufs=2, space=MS.PSUM) as psC,
        ):
            # ---------- constants ----------
            def load_const(dram, shape, dtype=F32):
                t = pp.tile(shape, dtype, tag=dram.name + "_sb")
                nc.sync.dma_start(t[:], dram[:])
                return t

            c_tripos = load_const(d_tripos, [C, C])
            c_trimid = load_const(d_trimid, [C, C])
            c_trirevs = load_const(d_trirevs, [C, C])
            c_onescp = load_const(d_onescp, [C, 128])
            c_onescol = load_const(d_onescol, [C, 1])
            c_masks = load_const(d_masks, [C, C])
            c_maskj = load_const(d_maskj, [C, C], U8)
            c_ident = load_const(d_ident, [128, 128])
            c_identb = load_const(d_identb, [128, 128], BF16)
            c_ones1 = load_const(d_ones1, [1, C])
            c_wf2 = load_const(d_wf2, [DV, HL * M])
            c_wg2 = load_const(d_wg2, [DV, HL * DV])
            c_bg2 = load_const(d_bg2, [1, HL * DV])
            c_cq = load_const(d_cq, [128, HL, KW])
            c_ck = load_const(d_ck, [128, HL, KW])
            c_cv = load_const(d_cv, [128, HL, KW])
            c_eps6 = pp.tile([C, 1], F32, tag="c_eps6")
            nc.gpsimd.memset(c_eps6[:], 1e-6)
            c_eps5 = pp.tile([C, 1], F32, tag="c_eps5")
            nc.gpsimd.memset(c_eps5[:], EPS)
            c_half = pp.tile([C, 1], F32, tag="c_half")
            nc.gpsimd.memset(c_half[:], 0.5)

            # ================= Phase P: projections + conv + silu ========
            qT = pp.tile([128, HL, T], BF16, tag="qT")
            kT = pp.tile([128, HL, T], BF16, tag="kT")
            vT = pp.tile([128, HL, T], BF16, tag="vT")
            wT = pp.tile([128, HL, T], BF16, tag="wT")
            betaTh = pp.tile([HL, T], F32, tag="betaTh")
            f1T = pp.tile([128, T], F32, tag="f1T")
            g1T = pp.tile([128, T], F32, tag="g1T")

            with (
                tc.tile_pool(name="projpool", bufs=1) as jp,
                tc.tile_pool(name="wload", bufs=1) as wp,
                tc.tile_pool(name="convscr", bufs=1) as cvp,
            ):
                xt_sb = jp.tile([128, NKT, T], BF16, tag="xt_sb")
                xtr = d_xt.rearrange("(k p) t -> k p t", p=128)
                for kt in range(NKT):
                    nc.sync.dma_start(xt_sb[:, kt, :], xtr[kt])

                def project_convT(d_w, c_cw, out_tile):
                    w_sb = wp.tile([128, NKT, HL * 128], BF16, tag="w_load")
                    wr = d_w.rearrange("(k p) c -> k p c", p=128)
                    for kt in range(NKT):
                        nc.sync.dma_start(w_sb[:, kt, :], wr[kt])
                    for ct in range(HL):
                        acc = []
                        for tt in range(2):
                            ps = psC.tile([128, 512], F32, tag="pp")
                            for kt in range(NKT):
                                nc.tensor.matmul(
                                    ps[:],
                                    w_sb[:, kt, ct * 128:(ct + 1) * 128],
                                    xt_sb[:, kt, tt * 512:(tt + 1) * 512],
                                    start=(kt == 0), stop=(kt == NKT - 1),
                                )
                            acc.append(ps)
                        # Drain PSUM to SBUF on Scalar (Copy is table-free),
                        # then conv taps split across DVE (half A) and Pool
                        # (half B) from SBUF. y[t] = sum_i w[i] x[t-3+i].
                        xs = cvp.tile([128, T], F32, tag="xs")
                        nc.scalar.copy(xs[:, 0:512], acc[0][:])
                        nc.scalar.copy(xs[:, 512:1024], acc[1][:])
                        cacc = cvp.tile([128, T], F32, tag="cacc")
                        # half A on DVE (scalar_tensor_tensor ok)
                        nc.vector.tensor_scalar_mul(
                            cacc[:, 0:512], xs[:, 0:512],
                            c_cw[:, ct, KW - 1:KW])
                        for i in range(KW - 1):
                            d = KW - 1 - i  # left shift amount
                            nc.vector.scalar_tensor_tensor(
                                cacc[:, d:512], xs[:, 0:512 - d],
                                c_cw[:, ct, i:i + 1], cacc[:, d:512],
                                op0=ALU.mult, op1=ALU.add)
                        # half B on Pool (no stt there: mult into tmp, add)
                        nc.gpsimd.tensor_scalar_mul(
                            cacc[:, 512:1024], xs[:, 512:1024],
                            c_cw[:, ct, KW - 1:KW])
                        for i in range(KW - 1):
                            d = KW - 1 - i
                            ctmp = scr.tile([128, 512], F32, tag="outsb",
                                            bufs=2)
                            nc.gpsimd.tensor_scalar_mul(
                                ctmp[:], xs[:, 512 - d:1024 - d],
                                c_cw[:, ct, i:i + 1])
                            nc.gpsimd.tensor_add(cacc[:, 512:1024],
                                                 cacc[:, 512:1024], ctmp[:])
                        nc.scalar.activation(out_tile[:, ct, :], cacc[:],
                                             AF.Silu)

                project_convT(d_wq, c_cq, qT)
                project_convT(d_wk, c_ck, kT)
                project_convT(d_wv, c_cv, vT)
                project_convT(d_ww, c_cv, wT)

                # f1T / g1T projections (no conv)
                def proj128T(d_w, out):
                    w_sb = wp.tile([128, NKT, 128], BF16, tag="w_load")
                    wr = d_w.rearrange("(k p) c -> k p c", p=128)
                    for kt in range(NKT):
                        nc.sync.dma_start(w_sb[:, kt, :], wr[kt])
                    for tt in range(2):
                        ps = psC.tile([128, 512], F32, tag="pp")
                        for kt in range(NKT):
                            nc.tensor.matmul(
                                ps[:], w_sb[:, kt, :],
                                xt_sb[:, kt, tt * 512:(tt + 1) * 512],
                                start=(kt == 0), stop=(kt == NKT - 1))
                        nc.scalar.copy(out[:, tt * 512:(tt + 1) * 512], ps[:])

                proj128T(d_wf1, f1T)
                proj128T(d_wg1, g1T)

                # beta projection -> tanh(0.5 x) channel-major [HL, T]
                wb_sb = wp.tile([128, NKT, HL], BF16, tag="wb_load")
                wbr = d_wb.rearrange("(k p) c -> k p c", p=128)
                for kt in range(NKT):
                    nc.sync.dma_start(wb_sb[:, kt, :], wbr[kt])
                for tt in range(2):
                    ps = psC.tile([HL, 512], F32, tag="pp")
                    for kt in range(NKT):
                        nc.tensor.matmul(
                            ps[:], wb_sb[:, kt, :],
                            xt_sb[:, kt, tt * 512:(tt + 1) * 512],
                            start=(kt == 0), stop=(kt == NKT - 1))
                    nc.scalar.activation(betaTh[:, tt * 512:(tt + 1) * 512],
                                         ps[:], AF.Tanh, scale=0.5)

            # ---------- Wo to SBUF (after proj pools release) ----------
            wo_sb = pp.tile([128, HL, HID], BF16, tag="wo_sb")
            wor = d_wo.rearrange("(h p) o -> h p o", p=128)
            for h in range(HL):
                nc.sync.dma_start(wo_sb[:, h, :], wor[h])

            # ================= Phase PRE-A =================
            # PE: gate matmuls + per-chunk transposes of w/k/v/beta.
            # Scalar: tanh group, square group, ln group, exp group.
            t1All = pp.tile([128, NCH, HL * M], F32, tag="t1All")
            lnlamAll = pp.tile([128, NCH, HL * M], F32, tag="lnlamAll")
            gateAll = pp.tile([128, NCH, HL * DV], BF16, tag="gateAll")
            wcT = pp.tile([128, NCH, HL * 128], F32, tag="wcT")
            KcAll = pp.tile([128, NCH, HL * 128], BF16, tag="KcAll")
            VcAll = pp.tile([128, NCH, HL * 128], BF16, tag="VcAll")
            btAll = pp.tile([128, NCH, HL], F32, tag="btAll")
            ssAll = pp.tile([128, 2 * NCH], F32, tag="ssAll")
            rsAll = pp.tile([128, 2 * NCH], F32, tag="rsAll")
            junk = pp.tile([128, 128], BF16, tag="junk")
            bwAll = pp.tile([128, NCH, HL * M], BF16, tag="bwAll")
            LamAll = pp.tile([128, NCH, HL * M], BF16, tag="LamAll")
            EposAll = pp.tile([128, NCH, HL * M], BF16, tag="EposAll")
            LamCbAll = pp.tile([128, NCH, HL * M], BF16, tag="LamCbAll")
            LamCcAll = pp.tile([128, NCH, HL], F32, tag="LamCcAll")
            EnegAll = pp.tile([128, NCH, HL * M], BF16, tag="EnegAll")
            KdecAll = pp.tile([128, NCH, HL * M], BF16, tag="KdecAll")

            # --- Per chunk: PE gate matmuls + transposes; scalar tanh group;
            # --- Pool/Vector PSUM drains. (One loop: per-engine orders align.)
            for n in range(NCH):
                t0 = n * C
                gps = psA.tile([C, HL * M], F32, tag="pA")
                nc.tensor.matmul(gps[:], f1T[:, t0:t0 + C], c_wf2[:],
                                 start=True, stop=True)
                nc.scalar.activation(t1All[:, n, :], gps[:], AF.Tanh,
                                     scale=0.5)
                gt = psA.tile([C, HL * DV], F32, tag="pA")
                nc.tensor.matmul(gt[:], g1T[:, t0:t0 + C], c_wg2[:],
                                 start=True, stop=False)
                nc.tensor.matmul(gt[:], c_ones1[:], c_bg2[:],
                                 start=False, stop=True)
                tg = scr.tile([C, HL * DV], F32, tag="tg", bufs=2)
                nc.scalar.activation(tg[:], gt[:], AF.Tanh, scale=0.5)
                nc.vector.tensor_scalar(gateAll[:, n, :], tg[:],
                                        0.5, 0.5, op0=ALU.mult, op1=ALU.add)
                bt = psB.tile([C, HL], F32, tag="pB")
                nc.tensor.transpose(bt[:], betaTh[:, t0:t0 + C],
                                    c_ident[0:HL, 0:HL])
                nc.vector.tensor_copy(btAll[:, n, :], bt[:])
                for h in range(HL):
                    wp_ = psB.tile([C, 128], BF16, tag="pB")
                    nc.tensor.transpose(wp_[:], wT[:, h, t0:t0 + C], c_identb[:])
                    nc.vector.tensor_copy(
                        wcT[:, n, h * 128:(h + 1) * 128], wp_[:])
                    kps = psB.tile([C, 128], BF16, tag="pB")
                    nc.tensor.transpose(kps[:], kT[:, h, t0:t0 + C], c_identb[:])
                    nc.scalar.copy(
                        KcAll[:, n, h * 128:(h + 1) * 128], kps[:])
                    vps = psB.tile([C, 128], BF16, tag="pB")
                    nc.tensor.transpose(vps[:], vT[:, h, t0:t0 + C], c_identb[:])
                    nc.scalar.copy(
                        VcAll[:, n, h * 128:(h + 1) * 128], vps[:])

            # --- Scalar: square group (w sumsq) ---
            for n in range(NCH):
                for h in range(HL):
                    idx = n * HL + h
                    nc.scalar.activation(
                        junk[:], wcT[:, n, h * 128:(h + 1) * 128], AF.Square,
                        accum_out=ssAll[:, idx:idx + 1])

            # --- Scalar: Ln group (single batched instructions) ---
            nc.scalar.activation(lnlamAll[:, :, :], t1All[:, :, :], AF.Ln,
                                 scale=0.5, bias=c_half[:])
            sdAll = pp.tile([128, 2 * NCH], F32, tag="sdAll")
            nc.scalar.activation(sdAll[:], ssAll[:], AF.Ln, bias=c_eps6[:])

            # --- Scalar: Exp group (rs, cumsum decays) ---
            nc.scalar.activation(rsAll[:], sdAll[:], AF.Exp, scale=-0.5)
            # bw (vector): needs rs; interleave per chunk
            for n in range(NCH):
                for h in range(HL):
                    idx = n * HL + h
                    tmpb = scr.tile([C, 1], F32, tag="tmpb")
                    nc.vector.tensor_scalar(tmpb[:], btAll[:, n, h:h + 1],
                                            0.5, 0.5, op0=ALU.mult, op1=ALU.add)
                    rsb = scr.tile([C, 1], F32, tag="rsb")
                    nc.vector.tensor_mul(rsb[:], tmpb[:], rsAll[:, idx:idx + 1])
                    nc.vector.tensor_scalar_mul(
                        bwAll[:, n, h * 128:(h + 1) * 128],
                        wcT[:, n, h * 128:(h + 1) * 128], rsb[:])

            for n in range(NCH):
                lnl = lnlamAll[:, n, :]
                gc = psA.tile([C, HL * M], F32, tag="pA")
                nc.tensor.matmul(gc[:], c_tripos[:], lnl, start=True, stop=True)
                gcp = psA.tile([C, HL * M], F32, tag="pA")
                nc.tensor.matmul(gcp[:], c_trimid[:], lnl, start=True, stop=True)
                grev = psA.tile([C, HL * M], F32, tag="pA")
                nc.tensor.matmul(grev[:], c_trirevs[:], lnl, start=True,
                                 stop=True)
                lcb = psA.tile([128, HL * M], F32, tag="pA")
                nc.tensor.matmul(lcb[:], c_onescp[:], lnl, start=True, stop=True)
                lcc = psB.tile([M, HL], F32, tag="pB")
                for h in range(HL):
                    nc.tensor.matmul(lcc[:, h:h + 1],
                                     lnl[:, h * 128:(h + 1) * 128],
                                     c_onescol[:], start=True, stop=True)
                nc.scalar.activation(LamAll[:, n, :], gc[:], AF.Exp)
                nc.scalar.activation(EposAll[:, n, :], gcp[:], AF.Exp)
                ene = scr.tile([C, HL * M], BF16, tag="ene", bufs=2)
                nc.scalar.activation(ene[:], gcp[:], AF.Exp, scale=-1.0)
                erev = scr.tile([C, HL * M], BF16, tag="erev", bufs=2)
                nc.scalar.activation(erev[:], grev[:], AF.Exp)
                nc.scalar.activation(LamCbAll[:, n, :], lcb[:], AF.Exp)
                nc.scalar.activation(LamCcAll[:, n, :], lcc[:], AF.Exp)
                nc.vector.tensor_mul(EnegAll[:, n, :], ene[:], bwAll[:, n, :])
                nc.vector.tensor_mul(KdecAll[:, n, :], erev[:], bwAll[:, n, :])

            # ================= Phase PRE-B =================
            intraAll = pp.tile([128, NCH, HL * M], BF16, tag="intraAll")
            EnegTAll = pp.tile([128, NCH, HL * M], BF16, tag="EnegTAll")
            skkAll = pp.tile([128, NCH, HL * M], BF16, tag="skkAll")
            svkAll = pp.tile([128, NCH, HL * M], BF16, tag="svkAll")

            for n in range(NCH):
                t0 = n * C
                for h in range(HL):
                    hsl = slice(h * 128, (h + 1) * 128)
                    pt = psB.tile([C, C], F32, tag="pB")
                    nc.tensor.matmul(pt[:], kT[:, h, t0:t0 + C],
                                     qT[:, h, t0:t0 + C], start=True, stop=True)
                    ptm = scrB.tile([C, C], BF16, tag="ptm")
                    nc.vector.tensor_mul(ptm[:], pt[:], c_masks[:])
                    intra = psB.tile([C, M], F32, tag="pB")
                    nc.tensor.matmul(intra[:], ptm[:], EnegAll[:, n, hsl],
                                     start=True, stop=True)
                    nc.scalar.copy(intraAll[:, n, hsl], intra[:])
                    ent = psB.tile([M, C], BF16, tag="pB")
                    nc.tensor.transpose(ent[:], EnegAll[:, n, hsl], c_identb[:])
                    nc.vector.tensor_copy(EnegTAll[:, n, hsl], ent[:])
                    skk = psB.tile([DK, M], F32, tag="pB")
                    nc.tensor.matmul(skk[:], KcAll[:, n, hsl],
                                     KdecAll[:, n, hsl], start=True, stop=True)
                    nc.scalar.copy(skkAll[:, n, hsl], skk[:])
                    svk = psB.tile([M, DV], F32, tag="pB")
                    nc.tensor.matmul(svk[:], KdecAll[:, n, hsl],
                                     VcAll[:, n, hsl], start=True, stop=True)
                    nc.vector.tensor_copy(svkAll[:, n, hsl], svk[:])

            # ================= Phase SCAN =================
            Sk = [pp.tile([DK, M], F32, name=f"Sk{h}", tag=f"Sk{h}") for h in range(HL)]
            Sv = [pp.tile([M, DV], F32, name=f"Sv{h}", tag=f"Sv{h}") for h in range(HL)]
            SkB = [pp.tile([DK, M], BF16, name=f"SkB{h}", tag=f"SkB{h}") for h in range(HL)]
            SvB = [pp.tile([M, DV], BF16, name=f"SvB{h}", tag=f"SvB{h}") for h in range(HL)]
            for h in range(HL):
                nc.gpsimd.memset(Sk[h][:], 0.0)
                nc.gpsimd.memset(Sv[h][:], 0.0)
                nc.gpsimd.memset(SkB[h][:], 0.0)
                nc.gpsimd.memset(SvB[h][:], 0.0)
            oAll = pp.tile([128, NCH, HL * DV], F32, tag="oAll")

            for n in range(NCH):
                t0 = n * C
                for h in range(HL):
                    hsl = slice(h * 128, (h + 1) * 128)
                    qs = psB.tile([C, M], F32, tag="pB")
                    nc.tensor.matmul(qs[:], qT[:, h, t0:t0 + C], SkB[h][:],
                                     start=True, stop=True)
                    s1 = scrB.tile([C, M], F32, tag="s1")
                    nc.vector.scalar_tensor_tensor(
                        s1[:], qs[:], SCALE, LamAll[:, n, hsl],
                        op0=ALU.mult, op1=ALU.mult)
                    s2 = scrB.tile([C, M], F32, tag="s2")
                    nc.gpsimd.tensor_mul(s2[:], intraAll[:, n, hsl],
                                         EposAll[:, n, hsl])
                    sS = scrB.tile([C, M], F32, tag="sS")
                    nc.gpsimd.tensor_add(sS[:], s1[:], s2[:])
                    nmx = scrB.tile([C, 1], F32, tag="nmx")
                    nc.vector.tensor_reduce(nmx[:], sS[:], AX.X, ALU.max,
                                            negate=True)
                    pexp = scrB.tile([C, M], F32, tag="pexp")
                    den = scrB.tile([C, 1], F32, tag="den")
                    nc.scalar.activation(pexp[:], sS[:], AF.Exp, bias=nmx[:],
                                         accum_out=den[:])
                    rec = scrB.tile([C, 1], F32, tag="rec")
                    nc.vector.reciprocal(rec[:], den[:])
                    aN = scrB.tile([C, M], F32, tag="aN")
                    nc.gpsimd.tensor_scalar_mul(aN[:], pexp[:], rec[:])
                    aL = scrB.tile([C, M], BF16, tag="aL")
                    nc.vector.tensor_mul(aL[:], aN[:], LamAll[:, n, hsl])
                    aE = scrB.tile([C, M], BF16, tag="aE")
                    nc.gpsimd.tensor_mul(aE[:], aN[:], EposAll[:, n, hsl])
                    alt = psB.tile([M, C], BF16, tag="pB")
                    nc.tensor.transpose(alt[:], aL[:], c_identb[:])
                    aLT = scrB.tile([M, C], BF16, tag="aLT")
                    nc.vector.tensor_copy(aLT[:], alt[:])
                    aet = psB.tile([M, C], BF16, tag="pB")
                    nc.tensor.transpose(aet[:], aE[:], c_identb[:])
                    aET = scrB.tile([M, C], BF16, tag="aET")
                    nc.scalar.copy(aET[:], aet[:])
                    rt = psB.tile([C, C], F32, tag="pB")
                    nc.tensor.matmul(rt[:], EnegTAll[:, n, hsl], aET[:],
                                     start=True, stop=True)
                    rmt = scrB.tile([C, C], BF16, tag="rmt")
                    nc.gpsimd.memset(rmt[:], 0.0)
                    nc.vector.copy_predicated(rmt[:], c_maskj[:], rt[:])
                    o_ps = psB.tile([C, DV], F32, tag="pB")
                    nc.tensor.matmul(o_ps[:], aLT[:], SvB[h][:],
                                     start=True, stop=False)
                    nc.tensor.matmul(o_ps[:], rmt[:], VcAll[:, n, hsl],
                                     start=False, stop=True)
                    nc.scalar.copy(oAll[:, n, hsl], o_ps[:])
                    # state updates
                    skt = scrB.tile([DK, M], F32, tag="skt")
                    nc.gpsimd.tensor_mul(skt[:], Sk[h][:], LamCbAll[:, n, hsl])
                    nc.gpsimd.tensor_add(Sk[h][:], skt[:], skkAll[:, n, hsl])
                    nc.vector.scalar_tensor_tensor(
                        Sv[h][:], Sv[h][:], LamCcAll[:, n, h:h + 1],
                        svkAll[:, n, hsl], op0=ALU.mult, op1=ALU.add)
                    nc.gpsimd.tensor_copy(SkB[h][:], Sk[h][:])
                    nc.gpsimd.tensor_copy(SvB[h][:], Sv[h][:])

            # ================= Phase EPI + Wo =================
            ossAll = pp.tile([128, 2 * NCH], F32, tag="ossAll")
            odAll = pp.tile([128, 2 * NCH], F32, tag="odAll")
            orrAll = pp.tile([128, 2 * NCH], F32, tag="orrAll")
            oTAll = pp.tile([128, HL, T], BF16, tag="oTAll")

            for n in range(NCH):
                for h in range(HL):
                    idx = n * HL + h
                    nc.scalar.activation(
                        junk[:], oAll[:, n, h * 128:(h + 1) * 128], AF.Square,
                        accum_out=ossAll[:, idx:idx + 1])
            nc.scalar.activation(odAll[:], ossAll[:], AF.Ln,
                                 scale=1.0 / DV, bias=c_eps5[:])
            nc.scalar.activation(orrAll[:], odAll[:], AF.Exp, scale=-0.5)

            for n in range(NCH):
                o1 = scr.tile([C, HL * DV], F32, tag="o1", bufs=2)
                nc.vector.tensor_mul(o1[:], oAll[:, n, :], gateAll[:, n, :])
                for h in range(HL):
                    idx = n * HL + h
                    of = scr.tile([C, DV], BF16, tag="of")
                    nc.vector.tensor_scalar_mul(
                        of[:], o1[:, h * 128:(h + 1) * 128],
                        orrAll[:, idx:idx + 1])
                    ot = psB.tile([DV, C], BF16, tag="pB")
                    nc.tensor.transpose(ot[:], of[:], c_identb[:])
                    nc.scalar.copy(oTAll[:, h, n * C:(n + 1) * C], ot[:])
                # Wo for this 128-row block
                for cl in range(4):
                    ps = psC.tile([128, 512], F32, tag="pp")
                    for h in range(HL):
                        nc.tensor.matmul(
                            ps[:], oTAll[:, h, n * C:(n + 1) * C],
                            wo_sb[:, h, cl * 512:(cl + 1) * 512],
                            start=(h == 0), stop=(h == HL - 1))
                    osb = scr.tile([128, 512], F32, tag="outsb", bufs=2)
                    nc.scalar.copy(osb[:], ps[:])
                    nc.sync.dma_start(
                        d_out[n * 128:(n + 1) * 128, cl * 512:(cl + 1) * 512],
                        osb[:])
    nc.compile()
    return nc


def _host_inputs(inputs):
    f32 = np.float32
    bf16 = ml_dtypes.bfloat16
    X = np.ascontiguousarray(np.asarray(inputs["hidden_states"], f32)[0])
    XT = np.ascontiguousarray(X.T).astype(bf16)

    jj, ii = np.indices((C, C))
    tripos = (jj <= ii).astype(f32)
    trimid = ((jj <= ii).astype(f32) - (jj <= C // 2 - 1).astype(f32))
    trirevs = (jj > ii).astype(f32)
    onescp = np.ones((C, 128), f32)
    onescol = np.ones((C, 1), f32)
    masks = np.triu(np.full((C, C), SCALE, f32))   # [j,i] SCALE if j<=i
    maskj = np.triu(np.ones((C, C), np.uint8))
    ident = np.eye(128, dtype=f32)
    ones1 = np.ones((1, C), f32)

    Wo_full = np.asarray(inputs["Wo"], f32) * np.tile(
        np.asarray(inputs["norm_w"], f32), H)[:, None]

    in_maps = []
    for c in range(8):
        hsl = slice(c * HL * 128, (c + 1) * HL * 128)
        bsl = slice(c * HL, (c + 1) * HL)
        m = {
            "xt": XT,
            "wq": np.asarray(inputs["Wq"], f32)[:, hsl].astype(bf16),
            "wk": np.asarray(inputs["Wk"], f32)[:, hsl].astype(bf16),
            "wv": np.asarray(inputs["Wv"], f32)[:, hsl].astype(bf16),
            "ww": np.asarray(inputs["Ww"], f32)[:, hsl].astype(bf16),
            "wf1": np.asarray(inputs["Wf1"], f32).astype(bf16),
            "wg1": np.asarray(inputs["Wg1"], f32).astype(bf16),
            "wb": np.asarray(inputs["Wb"], f32)[:, bsl].astype(bf16),
            "wf2": np.ascontiguousarray(np.asarray(inputs["Wf2"], f32)[:, hsl]),
            "wg2": np.ascontiguousarray(np.asarray(inputs["Wg2"], f32)[:, hsl]),
            "bg2": np.ascontiguousarray(
                np.asarray(inputs["bg2"], f32)[None, hsl]),
            "wo": np.ascontiguousarray(Wo_full[hsl]).astype(bf16),
            "cq": np.ascontiguousarray(
                np.asarray(inputs["cq"], f32)[hsl].reshape(HL, 128, KW)
                .transpose(1, 0, 2)),
            "ck": np.ascontiguousarray(
                np.asarray(inputs["ck"], f32)[hsl].reshape(HL, 128, KW)
                .transpose(1, 0, 2)),
            "cv": np.ascontiguousarray(
                np.asarray(inputs["cv"], f32)[hsl].reshape(HL, 128, KW)
                .transpose(1, 0, 2)),
            "tripos": tripos, "trimid": trimid, "trirevs": trirevs,
            "onescp": onescp, "onescol": onescol,
            "masks": masks, "maskj": maskj,
            "ident": ident, "identb": ident.astype(bf16), "ones1": ones1,
        }
        in_maps.append(m)
    return in_maps


def kernel(_trace=False, **inputs):
    if "nc" not in _CACHE:
        _CACHE["nc"] = _build_nc()
    nc = _CACHE["nc"]
    in_maps = _host_inputs(inputs)
    res = run_bass_kernel_spmd(nc, in_maps, core_ids=list(range(8)),
                               trace=_trace)
    _CACHE["last_result"] = res
    out = np.zeros((T, HID), np.float32)
    for r in res.results:
        out += r["out"]
    return out.reshape(B, T, HID)


# revision 26
# speedup vs baseline: 2.1609x; 1.6192x over previous
"""GatedSlotAttention2 Trainium2 Bass kernel (optimized).

Sharding: 2 heads per core x 8 cores (H=16). Each core runs the full
pipeline for its two heads (projections -> causal conv+silu -> chunked
gated-slot scan -> RMSNorm-gate -> partial Wo matmul); the host sums the
8 partial outputs.

Key optimizations over the first working version:
- Chunk length C=128 (fills all SBUF partitions, halves instruction count).
- Scalar engine uses only {Silu, Tanh, Square, Ln, Exp, Copy}, emitted
  grouped by function family so the activation table is loaded ~5 times
  total instead of per-op (sigmoid/softplus are computed via tanh+ln).
- State-independent work (gates, cumsum decays, transposes, intra-chunk
  matmuls, state-update outer products) is hoisted out of the serial scan.
- bf16 matmul inputs everywhere validated safe (4x faster PE rows), f32
  kept for cumsums and state accumulation.
- PSUM->SBUF traffic spread across Pool/Vector engines; Scalar does only
  activations.
"""
import numpy as np
import ml_dtypes

import concourse.bass as bass
import concourse.bacc as bacc_mod
import concourse.mybir as mybir
import concourse.tile as tile
from concourse.bass_utils import run_bass_kernel_spmd

F32 = mybir.dt.float32
F32R = mybir.dt.float32r
BF16 = mybir.dt.bfloat16
U8 = mybir.dt.uint8
AF = mybir.ActivationFunctionType
ALU = mybir.AluOpType
MS = bass.MemorySpace
AX = mybir.AxisListType

B, T, HID = 1, 1024, 2048
H, DK, DV, M, KW = 16, 128, 128, 128, 4
SCALE = DK ** -0.5
EPS = 1e-5
C = 128           # chunk length
NCH = T // C      # 8 chunks
NKT = HID // 128  # 16 contraction tiles
HL = 2            # heads per core

_CACHE = {}


def _build_nc():
    nc = bacc_mod.Bacc("TRN2")

    # ---------------- DRAM I/O ----------------
    d_xt = nc.dram_tensor("xt", [HID, T], BF16, kind="ExternalInput")
    d_wq = nc.dram_tensor("wq", [HID, HL * DK], BF16, kind="ExternalInput")
    d_wk = nc.dram_tensor("wk", [HID, HL * DK], BF16, kind="ExternalInput")
    d_wv = nc.dram_tensor("wv", [HID, HL * DV], BF16, kind="ExternalInput")
    d_ww = nc.dram_tensor("ww", [HID, HL * M], BF16, kind="ExternalInput")
    d_wf1 = nc.dram_tensor("wf1", [HID, DV], BF16, kind="ExternalInput")
    d_wg1 = nc.dram_tensor("wg1", [HID, DV], BF16, kind="ExternalInput")
    d_wb = nc.dram_tensor("wb", [HID, HL], BF16, kind="ExternalInput")
    d_wf2 = nc.dram_tensor("wf2", [DV, HL * M], F32, kind="ExternalInput")
    d_wg2 = nc.dram_tensor("wg2", [DV, HL * DV], F32, kind="ExternalInput")
    d_bg2 = nc.dram_tensor("bg2", [1, HL * DV], F32, kind="ExternalInput")
    d_wo = nc.dram_tensor("wo", [HL * DV, HID], BF16, kind="ExternalInput")
    d_cdq = nc.dram_tensor("cdq", [128, HL * KW * 128], BF16,
                           kind="ExternalInput")
    d_cdk = nc.dram_tensor("cdk", [128, HL * KW * 128], BF16,
                           kind="ExternalInput")
    d_cdv = nc.dram_tensor("cdv", [128, HL * KW * 128], BF16,
                           kind="ExternalInput")
    # constants
    d_tripos = nc.dram_tensor("tripos", [C, C], F32, kind="ExternalInput")
    d_trimid = nc.dram_tensor("trimid", [C, C], F32, kind="ExternalInput")
    d_trirevs = nc.dram_tensor("trirevs", [C, C], F32, kind="ExternalInput")
    d_onescp = nc.dram_tensor("onescp", [C, 128], F32, kind="ExternalInput")
    d_onescol = nc.dram_tensor("onescol", [C, 1], F32, kind="ExternalInput")
    d_masks = nc.dram_tensor("masks", [C, C], F32, kind="ExternalInput")
    d_maskj = nc.dram_tensor("maskj", [C, C], U8, kind="ExternalInput")
    d_ident = nc.dram_tensor("ident", [128, 128], F32, kind="ExternalInput")
    d_identb = nc.dram_tensor("identb", [128, 128], BF16, kind="ExternalInput")
    d_ones1 = nc.dram_tensor("ones1", [1, C], F32, kind="ExternalInput")

    d_out = nc.dram_tensor("out", [T, HID], F32, kind="ExternalOutput")

    with tile.TileContext(nc) as tc:
        with (
            tc.tile_pool(name="persist", bufs=1) as pp,
            tc.tile_pool(name="scr", bufs=3) as scr,
            tc.tile_pool(name="scrB", bufs=3) as scrB,
            tc.tile_pool(name="psA", bufs=3, space=MS.PSUM) as psA,
            tc.tile_pool(name="psB", bufs=3, space=MS.PSUM) as psB,
            tc.tile_pool(name="psC", bufs=2, space=MS.PSUM) as psC,
        ):
            # ---------- constants ----------
            def load_const(dram, shape, dtype=F32):
                t = pp.tile(shape, dtype, tag=dram.name + "_sb")
                nc.sync.dma_start(t[:], dram[:])
                return t

            c_tripos = load_const(d_tripos, [C, C])
            c_trimid = load_const(d_trimid, [C, C])
            c_trirevs = load_const(d_trirevs, [C, C])
            c_onescp = load_const(d_onescp, [C, 128])
            c_onescol = load_const(d_onescol, [C, 1])
            c_masks = load_const(d_masks, [C, C])
            c_maskj = load_const(d_maskj, [C, C], U8)
            c_ident = load_const(d_ident, [128, 128])
            c_identb = load_const(d_identb, [128, 128], BF16)
            c_ones1 = load_const(d_ones1, [1, C])
            c_wf2 = load_const(d_wf2, [DV, HL * M])
            c_wg2 = load_const(d_wg2, [DV, HL * DV])
            c_bg2 = load_const(d_bg2, [1, HL * DV])
            c_cdq = load_const(d_cdq, [128, HL, KW, 128], BF16)
            c_cdk = load_const(d_cdk, [128, HL, KW, 128], BF16)
            c_cdv = load_const(d_cdv, [128, HL, KW, 128], BF16)
            c_eps6 = pp.tile([C, 1], F32, tag="c_eps6")
            nc.gpsimd.memset(c_eps6[:], 1e-6)
            c_eps5 = pp.tile([C, 1], F32, tag="c_eps5")
            nc.gpsimd.memset(c_eps5[:], EPS)
            c_half = pp.tile([C, 1], F32, tag="c_half")
            nc.gpsimd.memset(c_half[:], 0.5)

            # ================= Phase P: projections + conv + silu ========
            qT = pp.tile([128, HL, T], BF16, tag="qT")
            kT = pp.tile([128, HL, T], BF16, tag="kT")
            vT = pp.tile([128, HL, T], BF16, tag="vT")
            wT = pp.tile([128, HL, T], BF16, tag="wT")
            betaTh = pp.tile([HL, T], F32, tag="betaTh")
            f1T = pp.tile([128, T], F32, tag="f1T")
            g1T = pp.tile([128, T], F32, tag="g1T")

            with (
                tc.tile_pool(name="projpool", bufs=1) as jp,
                tc.tile_pool(name="wload", bufs=1) as wp,
                tc.tile_pool(name="convscr", bufs=1) as cvp,
            ):
                xt_sb = jp.tile([128, NKT, T], BF16, tag="xt_sb")
                xtr = d_xt.rearrange("(k p) t -> k p t", p=128)
                for kt in range(NKT):
                    nc.sync.dma_start(xt_sb[:, kt, :], xtr[kt])

                def project_convT(d_w, c_cd, out_tile):
                    w_sb = wp.tile([128, NKT, HL * 128], BF16, tag="w_load")
                    wr = d_w.rearrange("(k p) c -> k p c", p=128)
                    for kt in range(NKT):
                        nc.sync.dma_start(w_sb[:, kt, :], wr[kt])
                    for ct in range(HL):
                        acc = []
                        for tt in range(2):
                            ps = psC.tile([128, 512], F32, tag="pp")
                            for kt in range(NKT):
                                nc.tensor.matmul(
                                    ps[:],
                                    w_sb[:, kt, ct * 128:(ct + 1) * 128],
                                    xt_sb[:, kt, tt * 512:(tt + 1) * 512],
                                    start=(kt == 0), stop=(kt == NKT - 1),
                                )
                            acc.append(ps)
                        # Drain projection to SBUF bf16, then causal conv as
                        # PSUM-accumulated diag(w_i) matmuls; silu from PSUM.
                        xs = cvp.tile([128, T], BF16, tag="xs")
                        nc.scalar.copy(xs[:, 0:512], acc[0][:])
                        nc.scalar.copy(xs[:, 512:1024], acc[1][:])
                        for tt in range(2):
                            cps = psC.tile([128, 512], F32, tag="pp")
                            first = True
                            for i in range(KW - 1, -1, -1):
                                d = KW - 1 - i  # left shift amount
                                lhs = c_cd[:, ct, i, :]
                                if tt == 0:
                                    outap = cps[:, d:512]
                                    rhs = xs[:, 0:512 - d]
                                else:
                                    outap = cps[:]
                                    rhs = xs[:, 512 - d:1024 - d]
                                nc.tensor.matmul(
                                    outap, lhs, rhs,
                                    start=first, stop=(i == 0),
                                    skip_group_check=True)
                                first = False
                            nc.scalar.activation(
                                out_tile[:, ct, tt * 512:(tt + 1) * 512],
                                cps[:], AF.Silu)

                project_convT(d_wq, c_cdq, qT)
                project_convT(d_wk, c_cdk, kT)
                project_convT(d_wv, c_cdv, vT)
                project_convT(d_ww, c_cdv, wT)

                # f1T / g1T projections (no conv)
                def proj128T(d_w, out):
                    w_sb = wp.tile([128, NKT, 128], BF16, tag="w_load")
                    wr = d_w.rearrange("(k p) c -> k p c", p=128)
                    for kt in range(NKT):
                        nc.sync.dma_start(w_sb[:, kt, :], wr[kt])
                    for tt in range(2):
                        ps = psC.tile([128, 512], F32, tag="pp")
                        for kt in range(NKT):
                            nc.tensor.matmul(
                                ps[:], w_sb[:, kt, :],
                                xt_sb[:, kt, tt * 512:(tt + 1) * 512],
                                start=(kt == 0), stop=(kt == NKT - 1))
                        nc.scalar.copy(out[:, tt * 512:(tt + 1) * 512], ps[:])

                proj128T(d_wf1, f1T)
                proj128T(d_wg1, g1T)

                # beta projection -> tanh(0.5 x) channel-major [HL, T]
                wb_sb = wp.tile([128, NKT, HL], BF16, tag="wb_load")
                wbr = d_wb.rearrange("(k p) c -> k p c", p=128)
                for kt in range(NKT):
                    nc.sync.dma_start(wb_sb[:, kt, :], wbr[kt])
                for tt in range(2):
                    ps = psC.tile([HL, 512], F32, tag="pp")
                    for kt in range(NKT):
                        nc.tensor.matmul(
                            ps[:], wb_sb[:, kt, :],
                            xt_sb[:, kt, tt * 512:(tt + 1) * 512],
                            start=(kt == 0), stop=(kt == NKT - 1))
                    nc.scalar.activation(betaTh[:, tt * 512:(tt + 1) * 512],
                                         ps[:], AF.Tanh, scale=0.5)

            # ---------- Wo to SBUF (after proj pools release) ----------
            wo_sb = pp.tile([128, HL, HID], BF16, tag="wo_sb")
            wor = d_wo.rearrange("(h p) o -> h p o", p=128)
            for h in range(HL):
                nc.sync.dma_start(wo_sb[:, h, :], wor[h])

            # ================= Phase PRE-A =================
            # PE: gate matmuls + per-chunk transposes of w/k/v/beta.
            # Scalar: tanh group, square group, ln group, exp group.
            t1All = pp.tile([128, NCH, HL * M], F32, tag="t1All")
            lnlamAll = pp.tile([128, NCH, HL * M], F32, tag="lnlamAll")
            gateAll = pp.tile([128, NCH, HL * DV], BF16, tag="gateAll")
            wcT = pp.tile([128, NCH, HL * 128], F32, tag="wcT")
            KcAll = pp.tile([128, NCH, HL * 128], BF16, tag="KcAll")
            VcAll = pp.tile([128, NCH, HL * 128], BF16, tag="VcAll")
            btAll = pp.tile([128, NCH, HL], F32, tag="btAll")
            ssAll = pp.tile([128, 2 * NCH], F32, tag="ssAll")
            rsAll = pp.tile([128, 2 * NCH], F32, tag="rsAll")
            junk = pp.tile([128, 128], BF16, tag="junk")
            bwAll = pp.tile([128, NCH, HL * M], BF16, tag="bwAll")
            LamAll = pp.tile([128, NCH, HL * M], BF16, tag="LamAll")
            EposAll = pp.tile([128, NCH, HL * M], BF16, tag="EposAll")
            LamCbAll = pp.tile([128, NCH, HL * M], BF16, tag="LamCbAll")
            LamCcAll = pp.tile([128, NCH, HL], F32, tag="LamCcAll")
            EnegAll = pp.tile([128, NCH, HL * M], BF16, tag="EnegAll")
            KdecAll = pp.tile([128, NCH, HL * M], BF16, tag="KdecAll")

            # --- Per chunk: PE gate matmuls + transposes; scalar tanh group;
            # --- Pool/Vector PSUM drains. (One loop: per-engine orders align.)
            for n in range(NCH):
                t0 = n * C
                gps = psA.tile([C, HL * M], F32, tag="pA")
                nc.tensor.matmul(gps[:], f1T[:, t0:t0 + C], c_wf2[:],
                                 start=True, stop=True)
                nc.scalar.activation(t1All[:, n, :], gps[:], AF.Tanh,
                                     scale=0.5)
                gt = psA.tile([C, HL * DV], F32, tag="pA")
                nc.tensor.matmul(gt[:], g1T[:, t0:t0 + C], c_wg2[:],
                                 start=True, stop=False)
                nc.tensor.matmul(gt[:], c_ones1[:], c_bg2[:],
                                 start=False, stop=True)
                tg = scr.tile([C, HL * DV], F32, tag="tg", bufs=2)
                nc.scalar.activation(tg[:], gt[:], AF.Tanh, scale=0.5)
                nc.vector.tensor_scalar(gateAll[:, n, :], tg[:],
                                        0.5, 0.5, op0=ALU.mult, op1=ALU.add)
                bt = psB.tile([C, HL], F32, tag="pB")
                nc.tensor.transpose(bt[:], betaTh[:, t0:t0 + C],
                                    c_ident[0:HL, 0:HL])
                nc.vector.tensor_copy(btAll[:, n, :], bt[:])
                for h in range(HL):
                    wp_ = psB.tile([C, 128], BF16, tag="pB")
                    nc.tensor.transpose(wp_[:], wT[:, h, t0:t0 + C], c_identb[:])
                    nc.vector.tensor_copy(
                        wcT[:, n, h * 128:(h + 1) * 128], wp_[:])
                    kps = psB.tile([C, 128], BF16, tag="pB")
                    nc.tensor.transpose(kps[:], kT[:, h, t0:t0 + C], c_identb[:])
                    nc.scalar.copy(
                        KcAll[:, n, h * 128:(h + 1) * 128], kps[:])
                    vps = psB.tile([C, 128], BF16, tag="pB")
                    nc.tensor.transpose(vps[:], vT[:, h, t0:t0 + C], c_identb[:])
                    nc.scalar.copy(
                        VcAll[:, n, h * 128:(h + 1) * 128], vps[:])

            # --- Scalar: square group (w sumsq) ---
            for n in range(NCH):
                for h in range(HL):
                    idx = n * HL + h
                    nc.scalar.activation(
                        junk[:], wcT[:, n, h * 128:(h + 1) * 128], AF.Square,
                        accum_out=ssAll[:, idx:idx + 1])

            # --- Scalar: Ln group (single batched instructions) ---
            nc.scalar.activation(lnlamAll[:, :, :], t1All[:, :, :], AF.Ln,
                                 scale=0.5, bias=c_half[:])
            sdAll = pp.tile([128, 2 * NCH], F32, tag="sdAll")
            nc.scalar.activation(sdAll[:], ssAll[:], AF.Ln, bias=c_eps6[:])

            # --- Scalar: Exp group (rs, cumsum decays) ---
            nc.scalar.activation(rsAll[:], sdAll[:], AF.Exp, scale=-0.5)
            # bw (vector): needs rs; interleave per chunk
            for n in range(NCH):
                for h in range(HL):
                    idx = n * HL + h
                    tmpb = scr.tile([C, 1], F32, tag="tmpb")
                    nc.vector.tensor_scalar(tmpb[:], btAll[:, n, h:h + 1],
                                            0.5, 0.5, op0=ALU.mult, op1=ALU.add)
                    rsb = scr.tile([C, 1], F32, tag="rsb")
                    nc.vector.tensor_mul(rsb[:], tmpb[:], rsAll[:, idx:idx + 1])
                    nc.vector.tensor_scalar_mul(
                        bwAll[:, n, h * 128:(h + 1) * 128],
                        wcT[:, n, h * 128:(h + 1) * 128], rsb[:])

            for n in range(NCH):
                lnl = lnlamAll[:, n, :]
                gc = psA.tile([C, HL * M], F32, tag="pA")
                nc.tensor.matmul(gc[:], c_tripos[:], lnl, start=True, stop=True)
                gcp = psA.tile([C, HL * M], F32, tag="pA")
                nc.tensor.matmul(gcp[:], c_trimid[:], lnl, start=True, stop=True)
                grev = psA.tile([C, HL * M], F32, tag="pA")
                nc.tensor.matmul(grev[:], c_trirevs[:], lnl, start=True,
                                 stop=True)
                lcb = psA.tile([128, HL * M], F32, tag="pA")
                nc.tensor.matmul(lcb[:], c_onescp[:], lnl, start=True, stop=True)
                lcc = psB.tile([M, HL], F32, tag="pB")
                for h in range(HL):
                    nc.tensor.matmul(lcc[:, h:h + 1],
                                     lnl[:, h * 128:(h + 1) * 128],
                                     c_onescol[:], start=True, stop=True)
                nc.scalar.activation(LamAll[:, n, :], gc[:], AF.Exp)
                nc.scalar.activation(EposAll[:, n, :], gcp[:], AF.Exp)
                ene = scr.tile([C, HL * M], BF16, tag="ene", bufs=2)
                nc.scalar.activation(ene[:], gcp[:], AF.Exp, scale=-1.0)
                erev = scr.tile([C, HL * M], BF16, tag="erev", bufs=2)
                nc.scalar.activation(erev[:], grev[:], AF.Exp)
                nc.scalar.activation(LamCbAll[:, n, :], lcb[:], AF.Exp)
                nc.scalar.activation(LamCcAll[:, n, :], lcc[:], AF.Exp)
                nc.vector.tensor_mul(EnegAll[:, n, :], ene[:], bwAll[:, n, :])
                nc.vector.tensor_mul(KdecAll[:, n, :], erev[:], bwAll[:, n, :])

            # ================= Phase PRE-B =================
            intraAll = pp.tile([128, NCH, HL * M], BF16, tag="intraAll")
            EnegTAll = pp.tile([128, NCH, HL * M], BF16, tag="EnegTAll")
            skkAll = pp.tile([128, NCH, HL * M], BF16, tag="skkAll")
            svkAll = pp.tile([128, NCH, HL * M], BF16, tag="svkAll")

            for n in range(NCH):
                t0 = n * C
                for h in range(HL):
                    hsl = slice(h * 128, (h + 1) * 128)
                    pt = psB.tile([C, C], F32, tag="pB")
                    nc.tensor.matmul(pt[:], kT[:, h, t0:t0 + C],
                                     qT[:, h, t0:t0 + C], start=True, stop=True)
                    ptm = scrB.tile([C, C], BF16, tag="ptm")
                    nc.vector.tensor_mul(ptm[:], pt[:], c_masks[:])
                    intra = psB.tile([C, M], F32, tag="pB")
                    nc.tensor.matmul(intra[:], ptm[:], EnegAll[:, n, hsl],
                                     start=True, stop=True)
                    nc.scalar.copy(intraAll[:, n, hsl], intra[:])
                    ent = psB.tile([M, C], BF16, tag="pB")
                    nc.tensor.transpose(ent[:], EnegAll[:, n, hsl], c_identb[:])
                    nc.vector.tensor_copy(EnegTAll[:, n, hsl], ent[:])
                    skk = psB.tile([DK, M], F32, tag="pB")
                    nc.tensor.matmul(skk[:], KcAll[:, n, hsl],
                                     KdecAll[:, n, hsl], start=True, stop=True)
                    nc.scalar.copy(skkAll[:, n, hsl], skk[:])
                    svk = psB.tile([M, DV], F32, tag="pB")
                    nc.tensor.matmul(svk[:], KdecAll[:, n, hsl],
                                     VcAll[:, n, hsl], start=True, stop=True)
                    nc.vector.tensor_copy(svkAll[:, n, hsl], svk[:])

            # ================= Phase SCAN =================
            Sk = [pp.tile([DK, M], F32, name=f"Sk{h}", tag=f"Sk{h}") for h in range(HL)]
            Sv = [pp.tile([M, DV], F32, name=f"Sv{h}", tag=f"Sv{h}") for h in range(HL)]
            SkB = [pp.tile([DK, M], BF16, name=f"SkB{h}", tag=f"SkB{h}") for h in range(HL)]
            SvB = [pp.tile([M, DV], BF16, name=f"SvB{h}", tag=f"SvB{h}") for h in range(HL)]
            for h in range(HL):
                nc.gpsimd.memset(Sk[h][:], 0.0)
                nc.gpsimd.memset(Sv[h][:], 0.0)
                nc.gpsimd.memset(SkB[h][:], 0.0)
                nc.gpsimd.memset(SvB[h][:], 0.0)
            oAll = pp.tile([128, NCH, HL * DV], F32, tag="oAll")

            for n in range(NCH):
                t0 = n * C
                for h in range(HL):
                    hsl = slice(h * 128, (h + 1) * 128)
                    qs = psB.tile([C, M], F32, tag="pB")
                    nc.tensor.matmul(qs[:], qT[:, h, t0:t0 + C], SkB[h][:],
                                     start=True, stop=True)
                    s1 = scrB.tile([C, M], F32, tag="s1")
                    nc.vector.scalar_tensor_tensor(
                        s1[:], qs[:], SCALE, LamAll[:, n, hsl],
                        op0=ALU.mult, op1=ALU.mult)
                    s2 = scrB.tile([C, M], F32, tag="s2")
                    nc.vector.tensor_mul(s2[:], intraAll[:, n, hsl],
                                         EposAll[:, n, hsl])
                    sS = scrB.tile([C, M], F32, tag="sS")
                    nc.vector.tensor_add(sS[:], s1[:], s2[:])
                    nmx = scrB.tile([C, 1], F32, tag="nmx")
                    nc.vector.tensor_reduce(nmx[:], sS[:], AX.X, ALU.max,
                                            negate=True)
                    pexp = scrB.tile([C, M], F32, tag="pexp")
                    den = scrB.tile([C, 1], F32, tag="den")
                    nc.scalar.activation(pexp[:], sS[:], AF.Exp, bias=nmx[:],
                                         accum_out=den[:])
                    rec = scrB.tile([C, 1], F32, tag="rec")
                    nc.vector.reciprocal(rec[:], den[:])
                    aL = scrB.tile([C, M], BF16, tag="aL")
                    nc.vector.scalar_tensor_tensor(
                        aL[:], pexp[:], rec[:], LamAll[:, n, hsl],
                        op0=ALU.mult, op1=ALU.mult)
                    aE = scrB.tile([C, M], BF16, tag="aE")
                    nc.vector.scalar_tensor_tensor(
                        aE[:], pexp[:], rec[:], EposAll[:, n, hsl],
                        op0=ALU.mult, op1=ALU.mult)
                    alt = psB.tile([M, C], BF16, tag="pB")
                    nc.tensor.transpose(alt[:], aL[:], c_identb[:])
                    aLT = scrB.tile([M, C], BF16, tag="aLT")
                    nc.vector.tensor_copy(aLT[:], alt[:])
                    aet = psB.tile([M, C], BF16, tag="pB")
                    nc.tensor.transpose(aet[:], aE[:], c_identb[:])
                    aET = scrB.tile([M, C], BF16, tag="aET")
                    nc.scalar.copy(aET[:], aet[:])
                    rt = psB.tile([C, C], F32, tag="pB")
                    nc.tensor.matmul(rt[:], EnegTAll[:, n, hsl], aET[:],
                                     start=True, stop=True)
                    rmt = scrB.tile([C, C], BF16, tag="rmt")
                    nc.gpsimd.memset(rmt[:], 0.0)
                    nc.vector.copy_predicated(rmt[:], c_maskj[:], rt[:])
                    o_ps = psB.tile([C, DV], F32, tag="pB")
                    nc.tensor.matmul(o_ps[:], aLT[:], SvB[h][:],
                                     start=True, stop=False)
                    nc.tensor.matmul(o_ps[:], rmt[:], VcAll[:, n, hsl],
                                     start=False, stop=True)
                    nc.scalar.copy(oAll[:, n, hsl], o_ps[:])
                    # state updates
                    skt = scrB.tile([DK, M], F32, tag="skt")
                    nc.vector.tensor_mul(skt[:], Sk[h][:], LamCbAll[:, n, hsl])
                    nc.vector.tensor_add(Sk[h][:], skt[:], skkAll[:, n, hsl])
                    nc.vector.scalar_tensor_tensor(
                        Sv[h][:], Sv[h][:], LamCcAll[:, n, h:h + 1],
                        svkAll[:, n, hsl], op0=ALU.mult, op1=ALU.add)
                    nc.scalar.copy(SkB[h][:], Sk[h][:])
                    nc.scalar.copy(SvB[h][:], Sv[h][:])

            # ================= Phase EPI + Wo =================
            ossAll = pp.tile([128, 2 * NCH], F32, tag="ossAll")
            odAll = pp.tile([128, 2 * NCH], F32, tag="odAll")
            orrAll = pp.tile([128, 2 * NCH], F32, tag="orrAll")
            oTAll = pp.tile([128, HL, T], BF16, tag="oTAll")

            for n in range(NCH):
                for h in range(HL):
                    idx = n * HL + h
                    nc.scalar.activation(
                        junk[:], oAll[:, n, h * 128:(h + 1) * 128], AF.Square,
                        accum_out=ossAll[:, idx:idx + 1])
            nc.scalar.activation(odAll[:], ossAll[:], AF.Ln,
                                 scale=1.0 / DV, bias=c_eps5[:])
            nc.scalar.activation(orrAll[:], odAll[:], AF.Exp, scale=-0.5)

            for n in range(NCH):
                o1 = scr.tile([C, HL * DV], F32, tag="o1", bufs=2)
                nc.vector.tensor_mul(o1[:], oAll[:, n, :], gateAll[:, n, :])
                for h in range(HL):
                    idx = n * HL + h
                    of = scr.tile([C, DV], BF16, tag="of")
                    nc.vector.tensor_scalar_mul(
                        of[:], o1[:, h * 128:(h + 1) * 128],
                        orrAll[:, idx:idx + 1])
                    ot = psB.tile([DV, C], BF16, tag="pB")
                    nc.tensor.transpose(ot[:], of[:], c_identb[:])
                    nc.scalar.copy(oTAll[:, h, n * C:(n + 1) * C], ot[:])
                # Wo for this 128-row block
                for cl in range(4):
                    ps = psC.tile([128, 512], F32, tag="pp")
                    for h in range(HL):
                        nc.tensor.matmul(
                            ps[:], oTAll[:, h, n * C:(n + 1) * C],
                            wo_sb[:, h, cl * 512:(cl + 1) * 512],
                            start=(h == 0), stop=(h == HL - 1))
                    osb = scr.tile([128, 512], F32, tag="outsb", bufs=2)
                    nc.scalar.copy(osb[:], ps[:])
                    nc.sync.dma_start(
                        d_out[n * 128:(n + 1) * 128, cl * 512:(cl + 1) * 512],
                        osb[:])
    nc.compile()
    return nc


def _conv_diags(cw):
    """cw: [HL*128, KW] -> [128, HL*KW*128] bf16 of diag(cw[ct*128:(ct+1)*128, i])."""
    bf16 = ml_dtypes.bfloat16
    out = np.zeros((128, HL, KW, 128), np.float32)
    for ct in range(HL):
        for i in range(KW):
            np.fill_diagonal(out[:, ct, i, :], cw[ct * 128:(ct + 1) * 128, i])
    return np.ascontiguousarray(out.reshape(128, HL * KW * 128)).astype(bf16)


def _host_inputs(inputs):
    f32 = np.float32
    bf16 = ml_dtypes.bfloat16
    X = np.ascontiguousarray(np.asarray(inputs["hidden_states"], f32)[0])
    XT = np.ascontiguousarray(X.T).astype(bf16)

    jj, ii = np.indices((C, C))
    tripos = (jj <= ii).astype(f32)
    trimid = ((jj <= ii).astype(f32) - (jj <= C // 2 - 1).astype(f32))
    trirevs = (jj > ii).astype(f32)
    onescp = np.ones((C, 128), f32)
    onescol = np.ones((C, 1), f32)
    masks = np.triu(np.full((C, C), SCALE, f32))   # [j,i] SCALE if j<=i
    maskj = np.triu(np.ones((C, C), np.uint8))
    ident = np.eye(128, dtype=f32)
    ones1 = np.ones((1, C), f32)

    Wo_full = np.asarray(inputs["Wo"], f32) * np.tile(
        np.asarray(inputs["norm_w"], f32), H)[:, None]

    in_maps = []
    for c in range(8):
        hsl = slice(c * HL * 128, (c + 1) * HL * 128)
        bsl = slice(c * HL, (c + 1) * HL)
        m = {
            "xt": XT,
            "wq": np.asarray(inputs["Wq"], f32)[:, hsl].astype(bf16),
            "wk": np.asarray(inputs["Wk"], f32)[:, hsl].astype(bf16),
            "wv": np.asarray(inputs["Wv"], f32)[:, hsl].astype(bf16),
            "ww": np.asarray(inputs["Ww"], f32)[:, hsl].astype(bf16),
            "wf1": np.asarray(inputs["Wf1"], f32).astype(bf16),
            "wg1": np.asarray(inputs["Wg1"], f32).astype(bf16),
            "wb": np.asarray(inputs["Wb"], f32)[:, bsl].astype(bf16),
            "wf2": np.ascontiguousarray(np.asarray(inputs["Wf2"], f32)[:, hsl]),
            "wg2": np.ascontiguousarray(np.asarray(inputs["Wg2"], f32)[:, hsl]),
            "bg2": np.ascontiguousarray(
                np.asarray(inputs["bg2"], f32)[None, hsl]),
            "wo": np.ascontiguousarray(Wo_full[hsl]).astype(bf16),
            "cdq": _conv_diags(np.asarray(inputs["cq"], f32)[hsl]),
            "cdk": _conv_diags(np.asarray(inputs["ck"], f32)[hsl]),
            "cdv": _conv_diags(np.asarray(inputs["cv"], f32)[hsl]),
            "tripos": tripos, "trimid": trimid, "trirevs": trirevs,
            "onescp": onescp, "onescol": onescol,
            "masks": masks, "maskj": maskj,
            "ident": ident, "identb": ident.astype(bf16), "ones1": ones1,
        }
        in_maps.append(m)
    return in_maps


def kernel(_trace=False, **inputs):
    if "nc" not in _CACHE:
        _CACHE["nc"] = _build_nc()
    nc = _CACHE["nc"]
    in_maps = _host_inputs(inputs)
    res = run_bass_kernel_spmd(nc, in_maps, core_ids=list(range(8)),
                               trace=_trace)
    _CACHE["last_result"] = res
    out = np.zeros((T, HID), np.float32)
    for r in res.results:
        out += r["out"]
    return out.reshape(B, T, HID)


# revision 28
# speedup vs baseline: 2.3812x; 1.1019x over previous
"""GatedSlotAttention2 Trainium2 Bass kernel (optimized).

Sharding: 2 heads per core x 8 cores (H=16). Each core runs the full
pipeline for its two heads (projections -> causal conv+silu -> chunked
gated-slot scan -> RMSNorm-gate -> partial Wo matmul); the host sums the
8 partial outputs.

Key optimizations over the first working version:
- Chunk length C=128 (fills all SBUF partitions, halves instruction count).
- Scalar engine uses only {Silu, Tanh, Square, Ln, Exp, Copy}, emitted
  grouped by function family so the activation table is loaded ~5 times
  total instead of per-op (sigmoid/softplus are computed via tanh+ln).
- State-independent work (gates, cumsum decays, transposes, intra-chunk
  matmuls, state-update outer products) is hoisted out of the serial scan.
- bf16 matmul inputs everywhere validated safe (4x faster PE rows), f32
  kept for cumsums and state accumulation.
- PSUM->SBUF traffic spread across Pool/Vector engines; Scalar does only
  activations.
"""
import numpy as np
import ml_dtypes

import concourse.bass as bass
import concourse.bacc as bacc_mod
import concourse.mybir as mybir
import concourse.tile as tile
from concourse.bass_utils import run_bass_kernel_spmd

F32 = mybir.dt.float32
F32R = mybir.dt.float32r
BF16 = mybir.dt.bfloat16
U8 = mybir.dt.uint8
AF = mybir.ActivationFunctionType
ALU = mybir.AluOpType
MS = bass.MemorySpace
AX = mybir.AxisListType

B, T, HID = 1, 1024, 2048
H, DK, DV, M, KW = 16, 128, 128, 128, 4
SCALE = DK ** -0.5
EPS = 1e-5
C = 128           # chunk length
NCH = T // C      # 8 chunks
NKT = HID // 128  # 16 contraction tiles
HL = 2            # heads per core

_CACHE = {}


def _build_nc():
    nc = bacc_mod.Bacc("TRN2")

    # ---------------- DRAM I/O ----------------
    d_xt = nc.dram_tensor("xt", [HID, T], BF16, kind="ExternalInput")
    d_wq = nc.dram_tensor("wq", [HID, HL * DK], BF16, kind="ExternalInput")
    d_wk = nc.dram_tensor("wk", [HID, HL * DK], BF16, kind="ExternalInput")
    d_wv = nc.dram_tensor("wv", [HID, HL * DV], BF16, kind="ExternalInput")
    d_ww = nc.dram_tensor("ww", [HID, HL * M], BF16, kind="ExternalInput")
    d_wf1 = nc.dram_tensor("wf1", [HID, DV], BF16, kind="ExternalInput")
    d_wg1 = nc.dram_tensor("wg1", [HID, DV], BF16, kind="ExternalInput")
    d_wb = nc.dram_tensor("wb", [HID, HL], BF16, kind="ExternalInput")
    d_wf2 = nc.dram_tensor("wf2", [DV, HL * M], F32, kind="ExternalInput")
    d_wg2 = nc.dram_tensor("wg2", [DV, HL * DV], F32, kind="ExternalInput")
    d_bg2 = nc.dram_tensor("bg2", [1, HL * DV], F32, kind="ExternalInput")
    d_wo = nc.dram_tensor("wo", [HL * DV, HID], BF16, kind="ExternalInput")
    d_cdq = nc.dram_tensor("cdq", [128, HL * KW * 128], BF16,
                           kind="ExternalInput")
    d_cdk = nc.dram_tensor("cdk", [128, HL * KW * 128], BF16,
                           kind="ExternalInput")
    d_cdv = nc.dram_tensor("cdv", [128, HL * KW * 128], BF16,
                           kind="ExternalInput")
    # constants
    d_tripos = nc.dram_tensor("tripos", [C, C], F32, kind="ExternalInput")
    d_trimid = nc.dram_tensor("trimid", [C, C], F32, kind="ExternalInput")
    d_trirevs = nc.dram_tensor("trirevs", [C, C], F32, kind="ExternalInput")
    d_onescp = nc.dram_tensor("onescp", [C, 128], F32, kind="ExternalInput")
    d_onescol = nc.dram_tensor("onescol", [C, 1], F32, kind="ExternalInput")
    d_masks = nc.dram_tensor("masks", [C, C], F32, kind="ExternalInput")
    d_maskj = nc.dram_tensor("maskj", [C, C], U8, kind="ExternalInput")
    d_ident = nc.dram_tensor("ident", [128, 128], F32, kind="ExternalInput")
    d_identb = nc.dram_tensor("identb", [128, 128], BF16, kind="ExternalInput")
    d_ones1 = nc.dram_tensor("ones1", [1, C], F32, kind="ExternalInput")

    d_out = nc.dram_tensor("out", [T, HID], F32, kind="ExternalOutput")

    with tile.TileContext(nc) as tc:
        with (
            tc.tile_pool(name="persist", bufs=1) as pp,
            tc.tile_pool(name="scr", bufs=3) as scr,
            tc.tile_pool(name="scrB", bufs=3) as scrB,
            tc.tile_pool(name="psA", bufs=3, space=MS.PSUM) as psA,
            tc.tile_pool(name="psB", bufs=3, space=MS.PSUM) as psB,
            tc.tile_pool(name="psC", bufs=2, space=MS.PSUM) as psC,
        ):
            # ---------- constants ----------
            def load_const(dram, shape, dtype=F32):
                t = pp.tile(shape, dtype, tag=dram.name + "_sb")
                nc.sync.dma_start(t[:], dram[:])
                return t

            c_tripos = load_const(d_tripos, [C, C])
            c_trimid = load_const(d_trimid, [C, C])
            c_trirevs = load_const(d_trirevs, [C, C])
            c_onescp = load_const(d_onescp, [C, 128])
            c_onescol = load_const(d_onescol, [C, 1])
            c_masks = load_const(d_masks, [C, C])
            c_maskj = load_const(d_maskj, [C, C], U8)
            c_ident = load_const(d_ident, [128, 128])
            c_identb = load_const(d_identb, [128, 128], BF16)
            c_ones1 = load_const(d_ones1, [1, C])
            c_wf2 = load_const(d_wf2, [DV, HL * M])
            c_wg2 = load_const(d_wg2, [DV, HL * DV])
            c_bg2 = load_const(d_bg2, [1, HL * DV])
            c_cdq = load_const(d_cdq, [128, HL, KW, 128], BF16)
            c_cdk = load_const(d_cdk, [128, HL, KW, 128], BF16)
            c_cdv = load_const(d_cdv, [128, HL, KW, 128], BF16)
            c_eps6 = pp.tile([C, 1], F32, tag="c_eps6")
            nc.gpsimd.memset(c_eps6[:], 1e-6)
            c_eps5 = pp.tile([C, 1], F32, tag="c_eps5")
            nc.gpsimd.memset(c_eps5[:], EPS)
            c_half = pp.tile([C, 1], F32, tag="c_half")
            nc.gpsimd.memset(c_half[:], 0.5)

            # ================= Phase P: projections + conv + silu ========
            qT = pp.tile([128, HL, T], BF16, tag="qT")
            kT = pp.tile([128, HL, T], BF16, tag="kT")
            vT = pp.tile([128, HL, T], BF16, tag="vT")
            wT = pp.tile([128, HL, T], BF16, tag="wT")
            betaTh = pp.tile([HL, T], F32, tag="betaTh")
            f1T = pp.tile([128, T], F32, tag="f1T")
            g1T = pp.tile([128, T], F32, tag="g1T")

            with (
                tc.tile_pool(name="projpool", bufs=1) as jp,
                tc.tile_pool(name="wload", bufs=1) as wp,
                tc.tile_pool(name="convscr", bufs=1) as cvp,
            ):
                xt_sb = jp.tile([128, NKT, T], BF16, tag="xt_sb")
                xtr = d_xt.rearrange("(k p) t -> k p t", p=128)
                for kt in range(NKT):
                    nc.sync.dma_start(xt_sb[:, kt, :], xtr[kt])

                def project_convT(d_w, c_cd, out_tile):
                    w_sb = wp.tile([128, NKT, HL * 128], BF16, tag="w_load")
                    wr = d_w.rearrange("(k p) c -> k p c", p=128)
                    for kt in range(NKT):
                        nc.sync.dma_start(w_sb[:, kt, :], wr[kt])
                    for ct in range(HL):
                        acc = []
                        for tt in range(2):
                            ps = psC.tile([128, 512], F32, tag="pp")
                            for kt in range(NKT):
                                nc.tensor.matmul(
                                    ps[:],
                                    w_sb[:, kt, ct * 128:(ct + 1) * 128],
                                    xt_sb[:, kt, tt * 512:(tt + 1) * 512],
                                    start=(kt == 0), stop=(kt == NKT - 1),
                                )
                            acc.append(ps)
                        # Drain projection to SBUF bf16, then causal conv as
                        # PSUM-accumulated diag(w_i) matmuls; silu from PSUM.
                        xs = cvp.tile([128, T], BF16, tag="xs")
                        nc.scalar.copy(xs[:, 0:512], acc[0][:])
                        nc.scalar.copy(xs[:, 512:1024], acc[1][:])
                        for tt in range(2):
                            cps = psC.tile([128, 512], F32, tag="pp")
                            first = True
                            for i in range(KW - 1, -1, -1):
                                d = KW - 1 - i  # left shift amount
                                lhs = c_cd[:, ct, i, :]
                                if tt == 0:
                                    outap = cps[:, d:512]
                                    rhs = xs[:, 0:512 - d]
                                else:
                                    outap = cps[:]
                                    rhs = xs[:, 512 - d:1024 - d]
                                nc.tensor.matmul(
                                    outap, lhs, rhs,
                                    start=first, stop=(i == 0),
                                    skip_group_check=True)
                                first = False
                            nc.scalar.activation(
                                out_tile[:, ct, tt * 512:(tt + 1) * 512],
                                cps[:], AF.Silu)

                project_convT(d_wq, c_cdq, qT)
                project_convT(d_wk, c_cdk, kT)
                project_convT(d_wv, c_cdv, vT)
                project_convT(d_ww, c_cdv, wT)

                # f1T / g1T projections (no conv)
                def proj128T(d_w, out):
                    w_sb = wp.tile([128, NKT, 128], BF16, tag="w_load")
                    wr = d_w.rearrange("(k p) c -> k p c", p=128)
                    for kt in range(NKT):
                        nc.sync.dma_start(w_sb[:, kt, :], wr[kt])
                    for tt in range(2):
                        ps = psC.tile([128, 512], F32, tag="pp")
                        for kt in range(NKT):
                            nc.tensor.matmul(
                                ps[:], w_sb[:, kt, :],
                                xt_sb[:, kt, tt * 512:(tt + 1) * 512],
                                start=(kt == 0), stop=(kt == NKT - 1))
                        nc.scalar.copy(out[:, tt * 512:(tt + 1) * 512], ps[:])

                proj128T(d_wf1, f1T)
                proj128T(d_wg1, g1T)

                # beta projection -> tanh(0.5 x) channel-major [HL, T]
                wb_sb = wp.tile([128, NKT, HL], BF16, tag="wb_load")
                wbr = d_wb.rearrange("(k p) c -> k p c", p=128)
                for kt in range(NKT):
                    nc.sync.dma_start(wb_sb[:, kt, :], wbr[kt])
                for tt in range(2):
                    ps = psC.tile([HL, 512], F32, tag="pp")
                    for kt in range(NKT):
                        nc.tensor.matmul(
                            ps[:], wb_sb[:, kt, :],
                            xt_sb[:, kt, tt * 512:(tt + 1) * 512],
                            start=(kt == 0), stop=(kt == NKT - 1))
                    nc.scalar.activation(betaTh[:, tt * 512:(tt + 1) * 512],
                                         ps[:], AF.Tanh, scale=0.5)

            # fold the attention scale into q once
            nc.vector.tensor_scalar_mul(qT[:, :, :], qT[:, :, :], SCALE)

            # ---------- Wo to SBUF (after proj pools release) ----------
            wo_sb = pp.tile([128, HL, HID], BF16, tag="wo_sb")
            wor = d_wo.rearrange("(h p) o -> h p o", p=128)
            for h in range(HL):
                nc.sync.dma_start(wo_sb[:, h, :], wor[h])

            # ================= Phase PRE-A =================
            # PE: gate matmuls + per-chunk transposes of w/k/v/beta.
            # Scalar: tanh group, square group, ln group, exp group.
            t1All = pp.tile([128, NCH, HL * M], F32, tag="t1All")
            lnlamAll = pp.tile([128, NCH, HL * M], F32, tag="lnlamAll")
            gateAll = pp.tile([128, NCH, HL * DV], BF16, tag="gateAll")
            wcT = pp.tile([128, NCH, HL * 128], F32, tag="wcT")
            KcAll = pp.tile([128, NCH, HL * 128], BF16, tag="KcAll")
            VcAll = pp.tile([128, NCH, HL * 128], BF16, tag="VcAll")
            btAll = pp.tile([128, NCH, HL], F32, tag="btAll")
            ssAll = pp.tile([128, 2 * NCH], F32, tag="ssAll")
            rsAll = pp.tile([128, 2 * NCH], F32, tag="rsAll")
            junk = pp.tile([128, 128], BF16, tag="junk")
            bwAll = pp.tile([128, NCH, HL * M], BF16, tag="bwAll")
            LamAll = pp.tile([128, NCH, HL * M], BF16, tag="LamAll")
            EposAll = pp.tile([128, NCH, HL * M], BF16, tag="EposAll")
            LamCbAll = pp.tile([128, NCH, HL * M], BF16, tag="LamCbAll")
            LamCcAll = pp.tile([128, NCH, HL], F32, tag="LamCcAll")
            EnegAll = pp.tile([128, NCH, HL * M], BF16, tag="EnegAll")
            KdecAll = pp.tile([128, NCH, HL * M], BF16, tag="KdecAll")

            # --- Per chunk: PE gate matmuls + transposes; scalar tanh group;
            # --- Pool/Vector PSUM drains. (One loop: per-engine orders align.)
            for n in range(NCH):
                t0 = n * C
                gps = psA.tile([C, HL * M], F32, tag="pA")
                nc.tensor.matmul(gps[:], f1T[:, t0:t0 + C], c_wf2[:],
                                 start=True, stop=True)
                nc.scalar.activation(t1All[:, n, :], gps[:], AF.Tanh,
                                     scale=0.5)
                gt = psA.tile([C, HL * DV], F32, tag="pA")
                nc.tensor.matmul(gt[:], g1T[:, t0:t0 + C], c_wg2[:],
                                 start=True, stop=False)
                nc.tensor.matmul(gt[:], c_ones1[:], c_bg2[:],
                                 start=False, stop=True)
                tg = scr.tile([C, HL * DV], F32, tag="tg", bufs=2)
                nc.scalar.activation(tg[:], gt[:], AF.Tanh, scale=0.5)
                nc.vector.tensor_scalar(gateAll[:, n, :], tg[:],
                                        0.5, 0.5, op0=ALU.mult, op1=ALU.add)
                bt = psB.tile([C, HL], F32, tag="pB")
                nc.tensor.transpose(bt[:], betaTh[:, t0:t0 + C],
                                    c_ident[0:HL, 0:HL])
                nc.vector.tensor_copy(btAll[:, n, :], bt[:])
                for h in range(HL):
                    wp_ = psB.tile([C, 128], BF16, tag="pB")
                    nc.tensor.transpose(wp_[:], wT[:, h, t0:t0 + C], c_identb[:])
                    nc.vector.tensor_copy(
                        wcT[:, n, h * 128:(h + 1) * 128], wp_[:])
                    kps = psB.tile([C, 128], BF16, tag="pB")
                    nc.tensor.transpose(kps[:], kT[:, h, t0:t0 + C], c_identb[:])
                    nc.scalar.copy(
                        KcAll[:, n, h * 128:(h + 1) * 128], kps[:])
                    vps = psB.tile([C, 128], BF16, tag="pB")
                    nc.tensor.transpose(vps[:], vT[:, h, t0:t0 + C], c_identb[:])
                    nc.scalar.copy(
                        VcAll[:, n, h * 128:(h + 1) * 128], vps[:])

            # --- Scalar: square group (w sumsq) ---
            for n in range(NCH):
                for h in range(HL):
                    idx = n * HL + h
                    nc.scalar.activation(
                        junk[:], wcT[:, n, h * 128:(h + 1) * 128], AF.Square,
                        accum_out=ssAll[:, idx:idx + 1])

            # --- Scalar: Ln group (single batched instructions) ---
            nc.scalar.activation(lnlamAll[:, :, :], t1All[:, :, :], AF.Ln,
                                 scale=0.5, bias=c_half[:])
            sdAll = pp.tile([128, 2 * NCH], F32, tag="sdAll")
            nc.scalar.activation(sdAll[:], ssAll[:], AF.Ln, bias=c_eps6[:])

            # --- Scalar: Exp group (rs, cumsum decays) ---
            nc.scalar.activation(rsAll[:], sdAll[:], AF.Exp, scale=-0.5)
            # bw (vector): needs rs; interleave per chunk
            for n in range(NCH):
                for h in range(HL):
                    idx = n * HL + h
                    tmpb = scr.tile([C, 1], F32, tag="tmpb")
                    nc.vector.tensor_scalar(tmpb[:], btAll[:, n, h:h + 1],
                                            0.5, 0.5, op0=ALU.mult, op1=ALU.add)
                    rsb = scr.tile([C, 1], F32, tag="rsb")
                    nc.vector.tensor_mul(rsb[:], tmpb[:], rsAll[:, idx:idx + 1])
                    nc.vector.tensor_scalar_mul(
                        bwAll[:, n, h * 128:(h + 1) * 128],
                        wcT[:, n, h * 128:(h + 1) * 128], rsb[:])

            for n in range(NCH):
                lnl = lnlamAll[:, n, :]
                gc = psA.tile([C, HL * M], F32, tag="pA")
                nc.tensor.matmul(gc[:], c_tripos[:], lnl, start=True, stop=True)
                gcp = psA.tile([C, HL * M], F32, tag="pA")
                nc.tensor.matmul(gcp[:], c_trimid[:], lnl, start=True, stop=True)
                grev = psA.tile([C, HL * M], F32, tag="pA")
                nc.tensor.matmul(grev[:], c_trirevs[:], lnl, start=True,
                                 stop=True)
                lcb = psA.tile([128, HL * M], F32, tag="pA")
                nc.tensor.matmul(lcb[:], c_onescp[:], lnl, start=True, stop=True)
                lcc = psB.tile([M, HL], F32, tag="pB")
                for h in range(HL):
                    nc.tensor.matmul(lcc[:, h:h + 1],
                                     lnl[:, h * 128:(h + 1) * 128],
                                     c_onescol[:], start=True, stop=True)
                nc.scalar.activation(LamAll[:, n, :], gc[:], AF.Exp)
                nc.scalar.activation(EposAll[:, n, :], gcp[:], AF.Exp)
                ene = scr.tile([C, HL * M], BF16, tag="ene", bufs=2)
                nc.scalar.activation(ene[:], gcp[:], AF.Exp, scale=-1.0)
                erev = scr.tile([C, HL * M], BF16, tag="erev", bufs=2)
                nc.scalar.activation(erev[:], grev[:], AF.Exp)
                nc.scalar.activation(LamCbAll[:, n, :], lcb[:], AF.Exp)
                nc.scalar.activation(LamCcAll[:, n, :], lcc[:], AF.Exp)
                nc.vector.tensor_mul(EnegAll[:, n, :], ene[:], bwAll[:, n, :])
                nc.vector.tensor_mul(KdecAll[:, n, :], erev[:], bwAll[:, n, :])

            # ================= Phase PRE-B =================
            s2All = pp.tile([128, NCH, HL * M], BF16, tag="s2All")
            EnegTAll = pp.tile([128, NCH, HL * M], BF16, tag="EnegTAll")
            skkAll = pp.tile([128, NCH, HL * M], BF16, tag="skkAll")
            svkAll = pp.tile([128, NCH, HL * M], BF16, tag="svkAll")

            for n in range(NCH):
                t0 = n * C
                for h in range(HL):
                    hsl = slice(h * 128, (h + 1) * 128)
                    pt = psB.tile([C, C], F32, tag="pB")
                    nc.tensor.matmul(pt[:], kT[:, h, t0:t0 + C],
                                     qT[:, h, t0:t0 + C], start=True, stop=True)
                    ptm = scrB.tile([C, C], BF16, tag="ptm")
                    nc.vector.tensor_mul(ptm[:], pt[:], c_masks[:])
                    intra = psB.tile([C, M], F32, tag="pB")
                    nc.tensor.matmul(intra[:], ptm[:], EnegAll[:, n, hsl],
                                     start=True, stop=True)
                    nc.vector.tensor_mul(s2All[:, n, hsl], intra[:],
                                         EposAll[:, n, hsl])
                    ent = psB.tile([M, C], BF16, tag="pB")
                    nc.tensor.transpose(ent[:], EnegAll[:, n, hsl], c_identb[:])
                    nc.vector.tensor_copy(EnegTAll[:, n, hsl], ent[:])
                    skk = psB.tile([DK, M], F32, tag="pB")
                    nc.tensor.matmul(skk[:], KcAll[:, n, hsl],
                                     KdecAll[:, n, hsl], start=True, stop=True)
                    nc.scalar.copy(skkAll[:, n, hsl], skk[:])
                    svk = psB.tile([M, DV], F32, tag="pB")
                    nc.tensor.matmul(svk[:], KdecAll[:, n, hsl],
                                     VcAll[:, n, hsl], start=True, stop=True)
                    nc.vector.tensor_copy(svkAll[:, n, hsl], svk[:])

            # ================= Phase SCAN =================
            Sk = [pp.tile([DK, M], F32, name=f"Sk{h}", tag=f"Sk{h}") for h in range(HL)]
            Sv = [pp.tile([M, DV], F32, name=f"Sv{h}", tag=f"Sv{h}") for h in range(HL)]
            SkB = [pp.tile([DK, M], BF16, name=f"SkB{h}", tag=f"SkB{h}") for h in range(HL)]
            SvB = [pp.tile([M, DV], BF16, name=f"SvB{h}", tag=f"SvB{h}") for h in range(HL)]
            for h in range(HL):
                nc.gpsimd.memset(Sk[h][:], 0.0)
                nc.gpsimd.memset(Sv[h][:], 0.0)
                nc.gpsimd.memset(SkB[h][:], 0.0)
                nc.gpsimd.memset(SvB[h][:], 0.0)
            oAll = pp.tile([128, NCH, HL * DV], F32, tag="oAll")

            denEps = pp.tile([128, 2 * NCH], F32, tag="denEps")
            for n in range(NCH):
                t0 = n * C
                HS = [slice(h * 128, (h + 1) * 128) for h in range(HL)]
                qs, sS, pexp, den, aL, aE = {}, {}, {}, {}, {}, {}
                aLT, aET, rmt, o_ps = {}, {}, {}, {}
                for h in range(HL):
                    qs[h] = psB.tile([C, M], F32, tag="pB", name=f"qs{n}_{h}")
                    nc.tensor.matmul(qs[h][:], qT[:, h, t0:t0 + C], SkB[h][:],
                                     start=True, stop=True)
                for h in range(HL):
                    v1 = scrB.tile([C, M], F32, tag="s1", name=f"v1{n}_{h}")
                    nc.vector.tensor_mul(v1[:], qs[h][:], LamAll[:, n, HS[h]])
                    sS[h] = scrB.tile([C, M], F32, tag="sS", name=f"sS{n}_{h}")
                    nc.vector.tensor_add(sS[h][:], v1[:], s2All[:, n, HS[h]])
                for h in range(HL):
                    pexp[h] = scrB.tile([C, M], F32, tag="pexp",
                                        name=f"pexp{n}_{h}")
                    den[h] = scrB.tile([C, 1], F32, tag="den",
                                       name=f"den{n}_{h}")
                    nc.scalar.activation(pexp[h][:], sS[h][:], AF.Exp,
                                         accum_out=den[h][:])
                for h in range(HL):
                    aL[h] = scrB.tile([C, M], BF16, tag="aL", name=f"aL{n}_{h}")
                    nc.vector.tensor_mul(aL[h][:], pexp[h][:],
                                         LamAll[:, n, HS[h]])
                    aE[h] = scrB.tile([C, M], BF16, tag="aE", name=f"aE{n}_{h}")
                    nc.vector.tensor_mul(aE[h][:], pexp[h][:],
                                         EposAll[:, n, HS[h]])
                for h in range(HL):
                    alt = psB.tile([M, C], BF16, tag="pB", name=f"alt{n}_{h}")
                    nc.tensor.transpose(alt[:], aL[h][:], c_identb[:])
                    aLT[h] = scrB.tile([M, C], BF16, tag="aLT",
                                       name=f"aLT{n}_{h}")
                    nc.vector.tensor_copy(aLT[h][:], alt[:])
                    aet = psB.tile([M, C], BF16, tag="pB", name=f"aet{n}_{h}")
                    nc.tensor.transpose(aet[:], aE[h][:], c_identb[:])
                    aET[h] = scrB.tile([M, C], BF16, tag="aET",
                                       name=f"aET{n}_{h}")
                    nc.scalar.copy(aET[h][:], aet[:])
                for h in range(HL):
                    rt = psB.tile([C, C], F32, tag="pB", name=f"rt{n}_{h}")
                    nc.tensor.matmul(rt[:], EnegTAll[:, n, HS[h]], aET[h][:],
                                     start=True, stop=True)
                    rmt[h] = scrB.tile([C, C], BF16, tag="rmt",
                                       name=f"rmt{n}_{h}")
                    nc.gpsimd.memset(rmt[h][:], 0.0)
                    nc.vector.copy_predicated(rmt[h][:], c_maskj[:], rt[:])
                for h in range(HL):
                    o_ps[h] = psB.tile([C, DV], F32, tag="pB",
                                       name=f"o{n}_{h}")
                    nc.tensor.matmul(o_ps[h][:], aLT[h][:], SvB[h][:],
                                     start=True, stop=False)
                    nc.tensor.matmul(o_ps[h][:], rmt[h][:], VcAll[:, n, HS[h]],
                                     start=False, stop=True)
                for h in range(HL):
                    nc.scalar.copy(oAll[:, n, HS[h]], o_ps[h][:])
                    idx = n * HL + h
                    nc.vector.scalar_tensor_tensor(
                        denEps[:, idx:idx + 1], den[h][:], EPS, den[h][:],
                        op0=ALU.mult, op1=ALU.mult)
                # state updates (inter-chunk serial chain)
                for h in range(HL):
                    skt = scrB.tile([DK, M], F32, tag="skt", name=f"skt{n}_{h}")
                    nc.vector.tensor_mul(skt[:], Sk[h][:], LamCbAll[:, n, HS[h]])
                    nc.vector.tensor_add(Sk[h][:], skt[:], skkAll[:, n, HS[h]])
                    nc.vector.scalar_tensor_tensor(
                        Sv[h][:], Sv[h][:], LamCcAll[:, n, h:h + 1],
                        svkAll[:, n, HS[h]], op0=ALU.mult, op1=ALU.add)
                    nc.gpsimd.tensor_copy(SkB[h][:], Sk[h][:])
                    nc.gpsimd.tensor_copy(SvB[h][:], Sv[h][:])

            # ================= Phase EPI + Wo =================
            # o here is den-scaled (softmax un-normalized); rsqrt absorbs it:
            # o/den / sqrt(mean((o/den)^2)+EPS) = o * rsqrt(oss/DV + EPS*den^2)
            ossAll = pp.tile([128, 2 * NCH], F32, tag="ossAll")
            zAll = pp.tile([128, 2 * NCH], F32, tag="zAll")
            odAll = pp.tile([128, 2 * NCH], F32, tag="odAll")
            orrAll = pp.tile([128, 2 * NCH], F32, tag="orrAll")
            oTAll = pp.tile([128, HL, T], BF16, tag="oTAll")

            for half in range(2):
                nsl = range(half * NCH // 2, (half + 1) * NCH // 2)
                isl = slice(half * NCH, (half + 1) * NCH)
                for n in nsl:
                    for h in range(HL):
                        idx = n * HL + h
                        nc.scalar.activation(
                            junk[:], oAll[:, n, h * 128:(h + 1) * 128],
                            AF.Square, accum_out=ossAll[:, idx:idx + 1])
                        nc.vector.scalar_tensor_tensor(
                            zAll[:, idx:idx + 1], ossAll[:, idx:idx + 1],
                            1.0 / DV, denEps[:, idx:idx + 1],
                            op0=ALU.mult, op1=ALU.add)
                nc.scalar.activation(odAll[:, isl], zAll[:, isl], AF.Ln)
                nc.scalar.activation(orrAll[:, isl], odAll[:, isl], AF.Exp,
                                     scale=-0.5)
                for n in nsl:
                    o1 = scr.tile([C, HL * DV], F32, tag="o1", bufs=2)
                    nc.vector.tensor_mul(o1[:], oAll[:, n, :], gateAll[:, n, :])
                    for h in range(HL):
                        idx = n * HL + h
                        of = scr.tile([C, DV], BF16, tag="of")
                        nc.vector.tensor_scalar_mul(
                            of[:], o1[:, h * 128:(h + 1) * 128],
                            orrAll[:, idx:idx + 1])
                        ot = psB.tile([DV, C], BF16, tag="pB")
                        nc.tensor.transpose(ot[:], of[:], c_identb[:])
                        nc.scalar.copy(oTAll[:, h, n * C:(n + 1) * C], ot[:])
                    # Wo for this 128-row block
                    for cl in range(4):
                        ps = psC.tile([128, 512], F32, tag="pp")
                        for h in range(HL):
                            nc.tensor.matmul(
                                ps[:], oTAll[:, h, n * C:(n + 1) * C],
                                wo_sb[:, h, cl * 512:(cl + 1) * 512],
                                start=(h == 0), stop=(h == HL - 1))
                        osb = scr.tile([128, 512], F32, tag="outsb", bufs=2)
                        nc.scalar.copy(osb[:], ps[:])
                        nc.sync.dma_start(
                            d_out[n * 128:(n + 1) * 128,
                                  cl * 512:(cl + 1) * 512],
                            osb[:])
    nc.compile()
    return nc


def _conv_diags(cw):
    """cw: [HL*128, KW] -> [128, HL*KW*128] bf16 of diag(cw[ct*128:(ct+1)*128, i])."""
    bf16 = ml_dtypes.bfloat16
    out = np.zeros((128, HL, KW, 128), np.float32)
    for ct in range(HL):
        for i in range(KW):
            np.fill_diagonal(out[:, ct, i, :], cw[ct * 128:(ct + 1) * 128, i])
    return np.ascontiguousarray(out.reshape(128, HL * KW * 128)).astype(bf16)


def _host_inputs(inputs):
    f32 = np.float32
    bf16 = ml_dtypes.bfloat16
    X = np.ascontiguousarray(np.asarray(inputs["hidden_states"], f32)[0])
    XT = np.ascontiguousarray(X.T).astype(bf16)

    jj, ii = np.indices((C, C))
    tripos = (jj <= ii).astype(f32)
    trimid = ((jj <= ii).astype(f32) - (jj <= C // 2 - 1).astype(f32))
    trirevs = (jj > ii).astype(f32)
    onescp = np.ones((C, 128), f32)
    onescol = np.ones((C, 1), f32)
    masks = np.triu(np.ones((C, C), f32))          # [j,i] 1 if j<=i
    maskj = np.triu(np.ones((C, C), np.uint8))
    ident = np.eye(128, dtype=f32)
    ones1 = np.ones((1, C), f32)

    Wo_full = np.asarray(inputs["Wo"], f32) * np.tile(
        np.asarray(inputs["norm_w"], f32), H)[:, None]

    in_maps = []
    for c in range(8):
        hsl = slice(c * HL * 128, (c + 1) * HL * 128)
        bsl = slice(c * HL, (c + 1) * HL)
        m = {
            "xt": XT,
            "wq": np.asarray(inputs["Wq"], f32)[:, hsl].astype(bf16),
            "wk": np.asarray(inputs["Wk"], f32)[:, hsl].astype(bf16),
            "wv": np.asarray(inputs["Wv"], f32)[:, hsl].astype(bf16),
            "ww": np.asarray(inputs["Ww"], f32)[:, hsl].astype(bf16),
            "wf1": np.asarray(inputs["Wf1"], f32).astype(bf16),
            "wg1": np.asarray(inputs["Wg1"], f32).astype(bf16),
            "wb": np.asarray(inputs["Wb"], f32)[:, bsl].astype(bf16),
            "wf2": np.ascontiguousarray(np.asarray(inputs["Wf2"], f32)[:, hsl]),
            "wg2": np.ascontiguousarray(np.asarray(inputs["Wg2"], f32)[:, hsl]),
            "bg2": np.ascontiguousarray(
                np.asarray(inputs["bg2"], f32)[None, hsl]),
            "wo": np.ascontiguousarray(Wo_full[hsl]).astype(bf16),
            "cdq": _conv_diags(np.asarray(inputs["cq"], f32)[hsl]),
            "cdk": _conv_diags(np.asarray(inputs["ck"], f32)[hsl]),
            "cdv": _conv_diags(np.asarray(inputs["cv"], f32)[hsl]),
            "tripos": tripos, "trimid": trimid, "trirevs": trirevs,
            "onescp": onescp, "onescol": onescol,
            "masks": masks, "maskj": maskj,
            "ident": ident, "identb": ident.astype(bf16), "ones1": ones1,
        }
        in_maps.append(m)
    return in_maps


def kernel(_trace=False, **inputs):
    if "nc" not in _CACHE:
        _CACHE["nc"] = _build_nc()
    nc = _CACHE["nc"]
    in_maps = _host_inputs(inputs)
    res = run_bass_kernel_spmd(nc, in_maps, core_ids=list(range(8)),
                               trace=_trace)
    _CACHE["last_result"] = res
    out = np.zeros((T, HID), np.float32)
    for r in res.results:
        out += r["out"]
    return out.reshape(B, T, HID)


# revision 29
# speedup vs baseline: 2.5315x; 1.0631x over previous
"""GatedSlotAttention2 Trainium2 Bass kernel (optimized).

Sharding: 2 heads per core x 8 cores (H=16). Each core runs the full
pipeline for its two heads (projections -> causal conv+silu -> chunked
gated-slot scan -> RMSNorm-gate -> partial Wo matmul); the host sums the
8 partial outputs.

Key optimizations over the first working version:
- Chunk length C=128 (fills all SBUF partitions, halves instruction count).
- Scalar engine uses only {Silu, Tanh, Square, Ln, Exp, Copy}, emitted
  grouped by function family so the activation table is loaded ~5 times
  total instead of per-op (sigmoid/softplus are computed via tanh+ln).
- State-independent work (gates, cumsum decays, transposes, intra-chunk
  matmuls, state-update outer products) is hoisted out of the serial scan.
- bf16 matmul inputs everywhere validated safe (4x faster PE rows), f32
  kept for cumsums and state accumulation.
- PSUM->SBUF traffic spread across Pool/Vector engines; Scalar does only
  activations.
"""
import numpy as np
import ml_dtypes

import concourse.bass as bass
import concourse.bacc as bacc_mod
import concourse.mybir as mybir
import concourse.tile as tile
from concourse.bass_utils import run_bass_kernel_spmd

F32 = mybir.dt.float32
F32R = mybir.dt.float32r
BF16 = mybir.dt.bfloat16
U8 = mybir.dt.uint8
AF = mybir.ActivationFunctionType
ALU = mybir.AluOpType
MS = bass.MemorySpace
AX = mybir.AxisListType

B, T, HID = 1, 1024, 2048
H, DK, DV, M, KW = 16, 128, 128, 128, 4
SCALE = DK ** -0.5
EPS = 1e-5
C = 128           # chunk length
NCH = T // C      # 8 chunks
NKT = HID // 128  # 16 contraction tiles
HL = 2            # heads per core

_CACHE = {}


def _build_nc():
    nc = bacc_mod.Bacc("TRN2")

    # ---------------- DRAM I/O ----------------
    d_xt = nc.dram_tensor("xt", [HID, T], BF16, kind="ExternalInput")
    d_wq = nc.dram_tensor("wq", [HID, HL * DK], BF16, kind="ExternalInput")
    d_wk = nc.dram_tensor("wk", [HID, HL * DK], BF16, kind="ExternalInput")
    d_wv = nc.dram_tensor("wv", [HID, HL * DV], BF16, kind="ExternalInput")
    d_ww = nc.dram_tensor("ww", [HID, HL * M], BF16, kind="ExternalInput")
    d_wf1 = nc.dram_tensor("wf1", [HID, DV], BF16, kind="ExternalInput")
    d_wg1 = nc.dram_tensor("wg1", [HID, DV], BF16, kind="ExternalInput")
    d_wb = nc.dram_tensor("wb", [HID, HL], BF16, kind="ExternalInput")
    d_wf2 = nc.dram_tensor("wf2", [DV, HL * M], F32, kind="ExternalInput")
    d_wg2 = nc.dram_tensor("wg2", [DV, HL * DV], F32, kind="ExternalInput")
    d_bg2 = nc.dram_tensor("bg2", [1, HL * DV], F32, kind="ExternalInput")
    d_wo = nc.dram_tensor("wo", [HL * DV, HID], BF16, kind="ExternalInput")
    d_cdq = nc.dram_tensor("cdq", [128, HL * KW * 128], BF16,
                           kind="ExternalInput")
    d_cdk = nc.dram_tensor("cdk", [128, HL * KW * 128], BF16,
                           kind="ExternalInput")
    d_cdv = nc.dram_tensor("cdv", [128, HL * KW * 128], BF16,
                           kind="ExternalInput")
    # constants
    d_tripos = nc.dram_tensor("tripos", [C, C], F32, kind="ExternalInput")
    d_trimid = nc.dram_tensor("trimid", [C, C], F32, kind="ExternalInput")
    d_trirevs = nc.dram_tensor("trirevs", [C, C], F32, kind="ExternalInput")
    d_onescp = nc.dram_tensor("onescp", [C, 128], F32, kind="ExternalInput")
    d_onescol = nc.dram_tensor("onescol", [C, 1], F32, kind="ExternalInput")
    d_masks = nc.dram_tensor("masks", [C, C], F32, kind="ExternalInput")
    d_maskj = nc.dram_tensor("maskj", [C, C], U8, kind="ExternalInput")
    d_ident = nc.dram_tensor("ident", [128, 128], F32, kind="ExternalInput")
    d_identb = nc.dram_tensor("identb", [128, 128], BF16, kind="ExternalInput")
    d_ones1 = nc.dram_tensor("ones1", [1, C], F32, kind="ExternalInput")

    d_out = nc.dram_tensor("out", [T, HID], BF16, kind="ExternalOutput")

    with tile.TileContext(nc) as tc:
        with (
            tc.tile_pool(name="persist", bufs=1) as pp,
            tc.tile_pool(name="scr", bufs=3) as scr,
            tc.tile_pool(name="scrB", bufs=3) as scrB,
            tc.tile_pool(name="psA", bufs=3, space=MS.PSUM) as psA,
            tc.tile_pool(name="psB", bufs=3, space=MS.PSUM) as psB,
            tc.tile_pool(name="psC", bufs=2, space=MS.PSUM) as psC,
        ):
            # ---------- constants ----------
            def load_const(dram, shape, dtype=F32):
                t = pp.tile(shape, dtype, tag=dram.name + "_sb")
                nc.sync.dma_start(t[:], dram[:])
                return t

            c_tripos = load_const(d_tripos, [C, C])
            c_trimid = load_const(d_trimid, [C, C])
            c_trirevs = load_const(d_trirevs, [C, C])
            c_onescp = load_const(d_onescp, [C, 128])
            c_onescol = load_const(d_onescol, [C, 1])
            c_masks = load_const(d_masks, [C, C])
            c_maskj = load_const(d_maskj, [C, C], U8)
            c_ident = load_const(d_ident, [128, 128])
            c_identb = load_const(d_identb, [128, 128], BF16)
            c_ones1 = load_const(d_ones1, [1, C])
            c_wf2 = load_const(d_wf2, [DV, HL * M])
            c_wg2 = load_const(d_wg2, [DV, HL * DV])
            c_bg2 = load_const(d_bg2, [1, HL * DV])
            c_cdq = load_const(d_cdq, [128, HL, KW, 128], BF16)
            c_cdk = load_const(d_cdk, [128, HL, KW, 128], BF16)
            c_cdv = load_const(d_cdv, [128, HL, KW, 128], BF16)
            c_eps6 = pp.tile([C, 1], F32, tag="c_eps6")
            nc.gpsimd.memset(c_eps6[:], 1e-6)
            c_eps5 = pp.tile([C, 1], F32, tag="c_eps5")
            nc.gpsimd.memset(c_eps5[:], EPS)
            c_half = pp.tile([C, 1], F32, tag="c_half")
            nc.gpsimd.memset(c_half[:], 0.5)

            # ================= Phase P: projections + conv + silu ========
            qT = pp.tile([128, HL, T], BF16, tag="qT")
            kT = pp.tile([128, HL, T], BF16, tag="kT")
            vT = pp.tile([128, HL, T], BF16, tag="vT")
            wT = pp.tile([128, HL, T], BF16, tag="wT")
            betaTh = pp.tile([HL, T], F32, tag="betaTh")
            f1T = pp.tile([128, T], F32, tag="f1T")
            g1T = pp.tile([128, T], F32, tag="g1T")

            with (
                tc.tile_pool(name="projpool", bufs=1) as jp,
                tc.tile_pool(name="wload", bufs=1) as wp,
                tc.tile_pool(name="convscr", bufs=1) as cvp,
            ):
                # first projection's weights before X so PE starts early
                wq_sb = wp.tile([128, NKT, HL * 128], BF16, tag="w_load")
                wqr = d_wq.rearrange("(k p) c -> k p c", p=128)
                for kt in range(NKT):
                    nc.sync.dma_start(wq_sb[:, kt, :], wqr[kt])

                xt_sb = jp.tile([128, NKT, T], BF16, tag="xt_sb")
                xtr = d_xt.rearrange("(k p) t -> k p t", p=128)
                for kt in range(NKT):
                    nc.sync.dma_start(xt_sb[:, kt, :], xtr[kt])

                def project_convT(d_w, c_cd, out_tile, w_sb=None):
                    if w_sb is None:
                        w_sb = wp.tile([128, NKT, HL * 128], BF16,
                                       tag="w_load")
                        wr = d_w.rearrange("(k p) c -> k p c", p=128)
                        for kt in range(NKT):
                            nc.sync.dma_start(w_sb[:, kt, :], wr[kt])
                    for ct in range(HL):
                        acc = []
                        for tt in range(2):
                            ps = psC.tile([128, 512], F32, tag="pp")
                            for kt in range(NKT):
                                nc.tensor.matmul(
                                    ps[:],
                                    w_sb[:, kt, ct * 128:(ct + 1) * 128],
                                    xt_sb[:, kt, tt * 512:(tt + 1) * 512],
                                    start=(kt == 0), stop=(kt == NKT - 1),
                                )
                            acc.append(ps)
                        # Drain projection to SBUF bf16, then causal conv as
                        # PSUM-accumulated diag(w_i) matmuls; silu from PSUM.
                        xs = cvp.tile([128, T], BF16, tag="xs")
                        nc.scalar.copy(xs[:, 0:512], acc[0][:])
                        nc.scalar.copy(xs[:, 512:1024], acc[1][:])
                        for tt in range(2):
                            cps = psC.tile([128, 512], F32, tag="pp")
                            first = True
                            for i in range(KW - 1, -1, -1):
                                d = KW - 1 - i  # left shift amount
                                lhs = c_cd[:, ct, i, :]
                                if tt == 0:
                                    outap = cps[:, d:512]
                                    rhs = xs[:, 0:512 - d]
                                else:
                                    outap = cps[:]
                                    rhs = xs[:, 512 - d:1024 - d]
                                nc.tensor.matmul(
                                    outap, lhs, rhs,
                                    start=first, stop=(i == 0),
                                    skip_group_check=True)
                                first = False
                            nc.scalar.activation(
                                out_tile[:, ct, tt * 512:(tt + 1) * 512],
                                cps[:], AF.Silu)

                project_convT(d_wq, c_cdq, qT, w_sb=wq_sb)
                project_convT(d_wk, c_cdk, kT)
                project_convT(d_wv, c_cdv, vT)
                project_convT(d_ww, c_cdv, wT)

                # f1T / g1T projections (no conv)
                def proj128T(d_w, out):
                    w_sb = wp.tile([128, NKT, 128], BF16, tag="w_load")
                    wr = d_w.rearrange("(k p) c -> k p c", p=128)
                    for kt in range(NKT):
                        nc.sync.dma_start(w_sb[:, kt, :], wr[kt])
                    for tt in range(2):
                        ps = psC.tile([128, 512], F32, tag="pp")
                        for kt in range(NKT):
                            nc.tensor.matmul(
                                ps[:], w_sb[:, kt, :],
                                xt_sb[:, kt, tt * 512:(tt + 1) * 512],
                                start=(kt == 0), stop=(kt == NKT - 1))
                        nc.scalar.copy(out[:, tt * 512:(tt + 1) * 512], ps[:])

                proj128T(d_wf1, f1T)
                proj128T(d_wg1, g1T)

                # beta projection -> tanh(0.5 x) channel-major [HL, T]
                wb_sb = wp.tile([128, NKT, HL], BF16, tag="wb_load")
                wbr = d_wb.rearrange("(k p) c -> k p c", p=128)
                for kt in range(NKT):
                    nc.sync.dma_start(wb_sb[:, kt, :], wbr[kt])
                for tt in range(2):
                    ps = psC.tile([HL, 512], F32, tag="pp")
                    for kt in range(NKT):
                        nc.tensor.matmul(
                            ps[:], wb_sb[:, kt, :],
                            xt_sb[:, kt, tt * 512:(tt + 1) * 512],
                            start=(kt == 0), stop=(kt == NKT - 1))
                    nc.scalar.activation(betaTh[:, tt * 512:(tt + 1) * 512],
                                         ps[:], AF.Tanh, scale=0.5)

            # fold the attention scale into q once
            nc.vector.tensor_scalar_mul(qT[:, :, :], qT[:, :, :], SCALE)

            # ---------- Wo to SBUF (after proj pools release) ----------
            wo_sb = pp.tile([128, HL, HID], BF16, tag="wo_sb")
            wor = d_wo.rearrange("(h p) o -> h p o", p=128)
            for h in range(HL):
                nc.sync.dma_start(wo_sb[:, h, :], wor[h])

            # ================= Phase PRE-A =================
            # PE: gate matmuls + per-chunk transposes of w/k/v/beta.
            # Scalar: tanh group, square group, ln group, exp group.
            t1All = pp.tile([128, NCH, HL * M], F32, tag="t1All")
            lnlamAll = pp.tile([128, NCH, HL * M], F32, tag="lnlamAll")
            gateAll = pp.tile([128, NCH, HL * DV], BF16, tag="gateAll")
            wcT = pp.tile([128, NCH, HL * 128], F32, tag="wcT")
            KcAll = pp.tile([128, NCH, HL * 128], BF16, tag="KcAll")
            VcAll = pp.tile([128, NCH, HL * 128], BF16, tag="VcAll")
            btAll = pp.tile([128, NCH, HL], F32, tag="btAll")
            ssAll = pp.tile([128, 2 * NCH], F32, tag="ssAll")
            rsAll = pp.tile([128, 2 * NCH], F32, tag="rsAll")
            junk = pp.tile([128, 128], BF16, tag="junk")
            bwAll = pp.tile([128, NCH, HL * M], BF16, tag="bwAll")
            LamAll = pp.tile([128, NCH, HL * M], BF16, tag="LamAll")
            EposAll = pp.tile([128, NCH, HL * M], BF16, tag="EposAll")
            LamCbAll = pp.tile([128, NCH, HL * M], BF16, tag="LamCbAll")
            LamCcAll = pp.tile([128, NCH, HL], F32, tag="LamCcAll")
            EnegAll = pp.tile([128, NCH, HL * M], BF16, tag="EnegAll")
            KdecAll = pp.tile([128, NCH, HL * M], BF16, tag="KdecAll")

            # --- Per chunk: PE gate matmuls + transposes; scalar tanh group;
            # --- Pool/Vector PSUM drains. (One loop: per-engine orders align.)
            for n in range(NCH):
                t0 = n * C
                gps = psA.tile([C, HL * M], F32, tag="pA")
                nc.tensor.matmul(gps[:], f1T[:, t0:t0 + C], c_wf2[:],
                                 start=True, stop=True)
                nc.scalar.activation(t1All[:, n, :], gps[:], AF.Tanh,
                                     scale=0.5)
                gt = psA.tile([C, HL * DV], F32, tag="pA")
                nc.tensor.matmul(gt[:], g1T[:, t0:t0 + C], c_wg2[:],
                                 start=True, stop=False)
                nc.tensor.matmul(gt[:], c_ones1[:], c_bg2[:],
                                 start=False, stop=True)
                tg = scr.tile([C, HL * DV], F32, tag="tg", bufs=2)
                nc.scalar.activation(tg[:], gt[:], AF.Tanh, scale=0.5)
                nc.vector.tensor_scalar(gateAll[:, n, :], tg[:],
                                        0.5, 0.5, op0=ALU.mult, op1=ALU.add)
                bt = psB.tile([C, HL], F32, tag="pB")
                nc.tensor.transpose(bt[:], betaTh[:, t0:t0 + C],
                                    c_ident[0:HL, 0:HL])
                nc.vector.tensor_copy(btAll[:, n, :], bt[:])
                for h in range(HL):
                    wp_ = psB.tile([C, 128], BF16, tag="pB")
                    nc.tensor.transpose(wp_[:], wT[:, h, t0:t0 + C], c_identb[:])
                    nc.vector.tensor_copy(
                        wcT[:, n, h * 128:(h + 1) * 128], wp_[:])
                    kps = psB.tile([C, 128], BF16, tag="pB")
                    nc.tensor.transpose(kps[:], kT[:, h, t0:t0 + C], c_identb[:])
                    nc.vector.tensor_copy(
                        KcAll[:, n, h * 128:(h + 1) * 128], kps[:])
                    vps = psB.tile([C, 128], BF16, tag="pB")
                    nc.tensor.transpose(vps[:], vT[:, h, t0:t0 + C], c_identb[:])
                    nc.vector.tensor_copy(
                        VcAll[:, n, h * 128:(h + 1) * 128], vps[:])

            # --- Scalar: square group (w sumsq) ---
            for n in range(NCH):
                for h in range(HL):
                    idx = n * HL + h
                    nc.scalar.activation(
                        junk[:], wcT[:, n, h * 128:(h + 1) * 128], AF.Square,
                        accum_out=ssAll[:, idx:idx + 1])

            # --- Scalar: Ln group (single batched instructions) ---
            nc.scalar.activation(lnlamAll[:, :, :], t1All[:, :, :], AF.Ln,
                                 scale=0.5, bias=c_half[:])
            sdAll = pp.tile([128, 2 * NCH], F32, tag="sdAll")
            nc.scalar.activation(sdAll[:], ssAll[:], AF.Ln, bias=c_eps6[:])

            # --- Scalar: Exp group (rs, cumsum decays) ---
            nc.scalar.activation(rsAll[:], sdAll[:], AF.Exp, scale=-0.5)
            # bw (vector): needs rs; interleave per chunk
            for n in range(NCH):
                for h in range(HL):
                    idx = n * HL + h
                    tmpb = scr.tile([C, 1], F32, tag="tmpb")
                    nc.vector.tensor_scalar(tmpb[:], btAll[:, n, h:h + 1],
                                            0.5, 0.5, op0=ALU.mult, op1=ALU.add)
                    rsb = scr.tile([C, 1], F32, tag="rsb")
                    nc.vector.tensor_mul(rsb[:], tmpb[:], rsAll[:, idx:idx + 1])
                    nc.vector.tensor_scalar_mul(
                        bwAll[:, n, h * 128:(h + 1) * 128],
                        wcT[:, n, h * 128:(h + 1) * 128], rsb[:])

            for n in range(NCH):
                lnl = lnlamAll[:, n, :]
                gc = psA.tile([C, HL * M], F32, tag="pA")
                nc.tensor.matmul(gc[:], c_tripos[:], lnl, start=True, stop=True)
                gcp = psA.tile([C, HL * M], F32, tag="pA")
                nc.tensor.matmul(gcp[:], c_trimid[:], lnl, start=True, stop=True)
                grev = psA.tile([C, HL * M], F32, tag="pA")
                nc.tensor.matmul(grev[:], c_trirevs[:], lnl, start=True,
                                 stop=True)
                lcb = psA.tile([128, HL * M], F32, tag="pA")
                nc.tensor.matmul(lcb[:], c_onescp[:], lnl, start=True, stop=True)
                lcc = psB.tile([M, HL], F32, tag="pB")
                for h in range(HL):
                    nc.tensor.matmul(lcc[:, h:h + 1],
                                     lnl[:, h * 128:(h + 1) * 128],
                                     c_onescol[:], start=True, stop=True)
                nc.scalar.activation(LamAll[:, n, :], gc[:], AF.Exp)
                nc.scalar.activation(EposAll[:, n, :], gcp[:], AF.Exp)
                ene = scr.tile([C, HL * M], BF16, tag="ene", bufs=2)
                nc.scalar.activation(ene[:], gcp[:], AF.Exp, scale=-1.0)
                erev = scr.tile([C, HL * M], BF16, tag="erev", bufs=2)
                nc.scalar.activation(erev[:], grev[:], AF.Exp)
                nc.scalar.activation(LamCbAll[:, n, :], lcb[:], AF.Exp)
                nc.scalar.activation(LamCcAll[:, n, :], lcc[:], AF.Exp)
                nc.vector.tensor_mul(EnegAll[:, n, :], ene[:], bwAll[:, n, :])
                nc.vector.tensor_mul(KdecAll[:, n, :], erev[:], bwAll[:, n, :])

            # ================= Phase PRE-B =================
            s2All = pp.tile([128, NCH, HL * M], BF16, tag="s2All")
            EnegTAll = pp.tile([128, NCH, HL * M], BF16, tag="EnegTAll")
            skkAll = pp.tile([128, NCH, HL * M], BF16, tag="skkAll")
            svkAll = pp.tile([128, NCH, HL * M], BF16, tag="svkAll")

            for n in range(NCH):
                t0 = n * C
                for h in range(HL):
                    hsl = slice(h * 128, (h + 1) * 128)
                    pt = psB.tile([C, C], F32, tag="pB")
                    nc.tensor.matmul(pt[:], kT[:, h, t0:t0 + C],
                                     qT[:, h, t0:t0 + C], start=True, stop=True)
                    ptm = scrB.tile([C, C], BF16, tag="ptm")
                    nc.vector.tensor_mul(ptm[:], pt[:], c_masks[:])
                    intra = psB.tile([C, M], F32, tag="pB")
                    nc.tensor.matmul(intra[:], ptm[:], EnegAll[:, n, hsl],
                                     start=True, stop=True)
                    nc.vector.tensor_mul(s2All[:, n, hsl], intra[:],
                                         EposAll[:, n, hsl])
                    ent = psB.tile([M, C], BF16, tag="pB")
                    nc.tensor.transpose(ent[:], EnegAll[:, n, hsl], c_identb[:])
                    nc.vector.tensor_copy(EnegTAll[:, n, hsl], ent[:])
                    skk = psB.tile([DK, M], F32, tag="pB")
                    nc.tensor.matmul(skk[:], KcAll[:, n, hsl],
                                     KdecAll[:, n, hsl], start=True, stop=True)
                    nc.scalar.copy(skkAll[:, n, hsl], skk[:])
                    svk = psB.tile([M, DV], F32, tag="pB")
                    nc.tensor.matmul(svk[:], KdecAll[:, n, hsl],
                                     VcAll[:, n, hsl], start=True, stop=True)
                    nc.vector.tensor_copy(svkAll[:, n, hsl], svk[:])

            # ================= Phase SCAN =================
            Sk = [pp.tile([DK, M], F32, name=f"Sk{h}", tag=f"Sk{h}") for h in range(HL)]
            Sv = [pp.tile([M, DV], F32, name=f"Sv{h}", tag=f"Sv{h}") for h in range(HL)]
            SkB = [pp.tile([DK, M], BF16, name=f"SkB{h}", tag=f"SkB{h}") for h in range(HL)]
            SvB = [pp.tile([M, DV], BF16, name=f"SvB{h}", tag=f"SvB{h}") for h in range(HL)]
            for h in range(HL):
                nc.gpsimd.memset(Sk[h][:], 0.0)
                nc.gpsimd.memset(Sv[h][:], 0.0)
                nc.gpsimd.memset(SkB[h][:], 0.0)
                nc.gpsimd.memset(SvB[h][:], 0.0)
            oAll = pp.tile([128, NCH, HL * DV], F32, tag="oAll")

            denEps = pp.tile([128, 2 * NCH], F32, tag="denEps")
            for n in range(NCH):
                t0 = n * C
                HS = [slice(h * 128, (h + 1) * 128) for h in range(HL)]
                qs, sS, pexp, den, aL, aE = {}, {}, {}, {}, {}, {}
                aLT, aET, rmt, o_ps = {}, {}, {}, {}
                for h in range(HL):
                    qs[h] = psB.tile([C, M], F32, tag="pB", name=f"qs{n}_{h}")
                    nc.tensor.matmul(qs[h][:], qT[:, h, t0:t0 + C], SkB[h][:],
                                     start=True, stop=True)
                for h in range(HL):
                    v1 = scrB.tile([C, M], F32, tag="s1", name=f"v1{n}_{h}")
                    nc.vector.tensor_mul(v1[:], qs[h][:], LamAll[:, n, HS[h]])
                    sS[h] = scrB.tile([C, M], F32, tag="sS", name=f"sS{n}_{h}")
                    nc.vector.tensor_add(sS[h][:], v1[:], s2All[:, n, HS[h]])
                for h in range(HL):
                    pexp[h] = scrB.tile([C, M], F32, tag="pexp",
                                        name=f"pexp{n}_{h}")
                    den[h] = scrB.tile([C, 1], F32, tag="den",
                                       name=f"den{n}_{h}")
                    nc.scalar.activation(pexp[h][:], sS[h][:], AF.Exp,
                                         accum_out=den[h][:])
                for h in range(HL):
                    aL[h] = scrB.tile([C, M], BF16, tag="aL", name=f"aL{n}_{h}")
                    nc.vector.tensor_mul(aL[h][:], pexp[h][:],
                                         LamAll[:, n, HS[h]])
                    aE[h] = scrB.tile([C, M], BF16, tag="aE", name=f"aE{n}_{h}")
                    nc.vector.tensor_mul(aE[h][:], pexp[h][:],
                                         EposAll[:, n, HS[h]])
                for h in range(HL):
                    alt = psB.tile([M, C], BF16, tag="pB", name=f"alt{n}_{h}")
                    nc.tensor.transpose(alt[:], aL[h][:], c_identb[:])
                    aLT[h] = scrB.tile([M, C], BF16, tag="aLT",
                                       name=f"aLT{n}_{h}")
                    nc.vector.tensor_copy(aLT[h][:], alt[:])
                    aet = psB.tile([M, C], BF16, tag="pB", name=f"aet{n}_{h}")
                    nc.tensor.transpose(aet[:], aE[h][:], c_identb[:])
                    aET[h] = scrB.tile([M, C], BF16, tag="aET",
                                       name=f"aET{n}_{h}")
                    nc.scalar.copy(aET[h][:], aet[:])
                for h in range(HL):
                    rt = psB.tile([C, C], F32, tag="pB", name=f"rt{n}_{h}")
                    nc.tensor.matmul(rt[:], EnegTAll[:, n, HS[h]], aET[h][:],
                                     start=True, stop=True)
                    rmt[h] = scrB.tile([C, C], BF16, tag="rmt",
                                       name=f"rmt{n}_{h}")
                    nc.gpsimd.memset(rmt[h][:], 0.0)
                    nc.vector.copy_predicated(rmt[h][:], c_maskj[:], rt[:])
                for h in range(HL):
                    o_ps[h] = psB.tile([C, DV], F32, tag="pB",
                                       name=f"o{n}_{h}")
                    nc.tensor.matmul(o_ps[h][:], aLT[h][:], SvB[h][:],
                                     start=True, stop=False)
                    nc.tensor.matmul(o_ps[h][:], rmt[h][:], VcAll[:, n, HS[h]],
                                     start=False, stop=True)
                for h in range(HL):
                    nc.scalar.copy(oAll[:, n, HS[h]], o_ps[h][:])
                    idx = n * HL + h
                    nc.vector.scalar_tensor_tensor(
                        denEps[:, idx:idx + 1], den[h][:], EPS, den[h][:],
                        op0=ALU.mult, op1=ALU.mult)
                # state updates (inter-chunk serial chain)
                for h in range(HL):
                    skt = scrB.tile([DK, M], F32, tag="skt", name=f"skt{n}_{h}")
                    nc.vector.tensor_mul(skt[:], Sk[h][:], LamCbAll[:, n, HS[h]])
                    nc.vector.tensor_add(Sk[h][:], skt[:], skkAll[:, n, HS[h]])
                    nc.vector.scalar_tensor_tensor(
                        Sv[h][:], Sv[h][:], LamCcAll[:, n, h:h + 1],
                        svkAll[:, n, HS[h]], op0=ALU.mult, op1=ALU.add)
                    nc.gpsimd.tensor_copy(SkB[h][:], Sk[h][:])
                    nc.gpsimd.tensor_copy(SvB[h][:], Sv[h][:])

            # ================= Phase EPI + Wo =================
            # o here is den-scaled (softmax un-normalized); rsqrt absorbs it:
            # o/den / sqrt(mean((o/den)^2)+EPS) = o * rsqrt(oss/DV + EPS*den^2)
            ossAll = pp.tile([128, 2 * NCH], F32, tag="ossAll")
            zAll = pp.tile([128, 2 * NCH], F32, tag="zAll")
            odAll = pp.tile([128, 2 * NCH], F32, tag="odAll")
            orrAll = pp.tile([128, 2 * NCH], F32, tag="orrAll")
            oTAll = pp.tile([128, HL, T], BF16, tag="oTAll")

            for half in range(2):
                nsl = range(half * NCH // 2, (half + 1) * NCH // 2)
                isl = slice(half * NCH, (half + 1) * NCH)
                for n in nsl:
                    for h in range(HL):
                        idx = n * HL + h
                        nc.scalar.activation(
                            junk[:], oAll[:, n, h * 128:(h + 1) * 128],
                            AF.Square, accum_out=ossAll[:, idx:idx + 1])
                        nc.vector.scalar_tensor_tensor(
                            zAll[:, idx:idx + 1], ossAll[:, idx:idx + 1],
                            1.0 / DV, denEps[:, idx:idx + 1],
                            op0=ALU.mult, op1=ALU.add)
                nc.scalar.activation(odAll[:, isl], zAll[:, isl], AF.Ln)
                nc.scalar.activation(orrAll[:, isl], odAll[:, isl], AF.Exp,
                                     scale=-0.5)
                for n in nsl:
                    o1 = scr.tile([C, HL * DV], F32, tag="o1", bufs=2)
                    nc.vector.tensor_mul(o1[:], oAll[:, n, :], gateAll[:, n, :])
                    for h in range(HL):
                        idx = n * HL + h
                        of = scr.tile([C, DV], BF16, tag="of")
                        nc.vector.tensor_scalar_mul(
                            of[:], o1[:, h * 128:(h + 1) * 128],
                            orrAll[:, idx:idx + 1])
                        ot = psB.tile([DV, C], BF16, tag="pB")
                        nc.tensor.transpose(ot[:], of[:], c_identb[:])
                        nc.vector.tensor_copy(oTAll[:, h, n * C:(n + 1) * C], ot[:])
                    # Wo for this 128-row block
                    for cl in range(4):
                        ps = psC.tile([128, 512], F32, tag="pp")
                        for h in range(HL):
                            nc.tensor.matmul(
                                ps[:], oTAll[:, h, n * C:(n + 1) * C],
                                wo_sb[:, h, cl * 512:(cl + 1) * 512],
                                start=(h == 0), stop=(h == HL - 1))
                        osb = scr.tile([128, 512], BF16, tag="outsb", bufs=2)
                        nc.vector.tensor_copy(osb[:], ps[:])
                        nc.sync.dma_start(
                            d_out[n * 128:(n + 1) * 128,
                                  cl * 512:(cl + 1) * 512],
                            osb[:])
    nc.compile()
    return nc


def _conv_diags(cw):
    """cw: [HL*128, KW] -> [128, HL*KW*128] bf16 of diag(cw[ct*128:(ct+1)*128, i])."""
    bf16 = ml_dtypes.bfloat16
    out = np.zeros((128, HL, KW, 128), np.float32)
    for ct in range(HL):
        for i in range(KW):
            np.fill_diagonal(out[:, ct, i, :], cw[ct * 128:(ct + 1) * 128, i])
    return np.ascontiguousarray(out.reshape(128, HL * KW * 128)).astype(bf16)


def _host_inputs(inputs):
    f32 = np.float32
    bf16 = ml_dtypes.bfloat16
    X = np.ascontiguousarray(np.asarray(inputs["hidden_states"], f32)[0])
    XT = np.ascontiguousarray(X.T).astype(bf16)

    jj, ii = np.indices((C, C))
    tripos = (jj <= ii).astype(f32)
    trimid = ((jj <= ii).astype(f32) - (jj <= C // 2 - 1).astype(f32))
    trirevs = (jj > ii).astype(f32)
    onescp = np.ones((C, 128), f32)
    onescol = np.ones((C, 1), f32)
    masks = np.triu(np.ones((C, C), f32))          # [j,i] 1 if j<=i
    maskj = np.triu(np.ones((C, C), np.uint8))
    ident = np.eye(128, dtype=f32)
    ones1 = np.ones((1, C), f32)

    Wo_full = np.asarray(inputs["Wo"], f32) * np.tile(
        np.asarray(inputs["norm_w"], f32), H)[:, None]

    in_maps = []
    for c in range(8):
        hsl = slice(c * HL * 128, (c + 1) * HL * 128)
        bsl = slice(c * HL, (c + 1) * HL)
        m = {
            "xt": XT,
            "wq": np.asarray(inputs["Wq"], f32)[:, hsl].astype(bf16),
            "wk": np.asarray(inputs["Wk"], f32)[:, hsl].astype(bf16),
            "wv": np.asarray(inputs["Wv"], f32)[:, hsl].astype(bf16),
            "ww": np.asarray(inputs["Ww"], f32)[:, hsl].astype(bf16),
            "wf1": np.asarray(inputs["Wf1"], f32).astype(bf16),
            "wg1": np.asarray(inputs["Wg1"], f32).astype(bf16),
            "wb": np.asarray(inputs["Wb"], f32)[:, bsl].astype(bf16),
            "wf2": np.ascontiguousarray(np.asarray(inputs["Wf2"], f32)[:, hsl]),
            "wg2": np.ascontiguousarray(np.asarray(inputs["Wg2"], f32)[:, hsl]),
            "bg2": np.ascontiguousarray(
                np.asarray(inputs["bg2"], f32)[None, hsl]),
            "wo": np.ascontiguousarray(Wo_full[hsl]).astype(bf16),
            "cdq": _conv_diags(np.asarray(inputs["cq"], f32)[hsl]),
            "cdk": _conv_diags(np.asarray(inputs["ck"], f32)[hsl]),
            "cdv": _conv_diags(np.asarray(inputs["cv"], f32)[hsl]),
            "tripos": tripos, "trimid": trimid, "trirevs": trirevs,
            "onescp": onescp, "onescol": onescol,
            "masks": masks, "maskj": maskj,
            "ident": ident, "identb": ident.astype(bf16), "ones1": ones1,
        }
        in_maps.append(m)
    return in_maps


def kernel(_trace=False, **inputs):
    if "nc" not in _CACHE:
        _CACHE["nc"] = _build_nc()
    nc = _CACHE["nc"]
    in_maps = _host_inputs(inputs)
    res = run_bass_kernel_spmd(nc, in_maps, core_ids=list(range(8)),
                               trace=_trace)
    _CACHE["last_result"] = res
    out = np.zeros((T, HID), np.float32)
    for r in res.results:
        out += np.asarray(r["out"], dtype=np.float32)
    return out.reshape(B, T, HID)
